# revision 1
# baseline (speedup 1.0000x reference)
"""Trainium2 kernel for nn_Circuit_41936060678727.

The reference is a 10-qubit real-amplitude circuit (CNOT ladders + RY
rotations) applied to an amplitude-embedded batch, measured with PauliZ on
each of the 10 wires.  Every gate is linear in the state, so the whole
8-layer circuit collapses to one fixed 1024x1024 orthogonal matrix M that
depends only on `params` (8x10).  With x padded to 1024 and L2-normalized:

    out[b, p] = sum_z (x[b] @ M[:784, :])[z]^2 * sign_p(z) / ||x[b]||^2

because M is orthogonal, ||x @ M[:784,:]|| = ||x||, so the norm is obtained
for free as an extra all-ones column in the sign matrix.

Device work per core (batch 16384 data-parallel over 8 cores, 2048 each):
    yT  [1024, 2048] = W^T @ xT          (tensor engine, f32r, K=784)
    sq  = yT^2                            (scalar engine)
    oT  [11, 2048]  = Zaug^T @ sq         (tensor engine, f32r, K=1024)
Host: out = (oT[:10] / oT[10]) ^T, concat cores.
"""

import numpy as np

N_QUBITS = 10
DIM = 1 << N_QUBITS          # 1024
N_OUT = 10
D_IN = 784
B_TOTAL = 16384
N_CORES = 8
B_CORE = B_TOTAL // N_CORES  # 2048
GROUP = 512                  # batch columns per matmul (one PSUM bank, fp32)
N_GROUPS = B_CORE // GROUP   # 4
K_CHUNK = 112                # 784 = 7 * 112 contraction chunks
N_KCH = D_IN // K_CHUNK      # 7
Z_CHUNK = 128
N_ZCH = DIM // Z_CHUNK       # 8
ZCOLS = 16                   # cols 0..9 = PauliZ signs, 10 = ones, 11..15 pad


# ----------------------------------------------------------------------------
# Host-side precompute: collapse the circuit to W = M[:784, :]
# ----------------------------------------------------------------------------

def _apply_ry(S, theta, q):
    B = S.shape[0]
    left, right = 1 << q, 1 << (N_QUBITS - q - 1)
    s = S.reshape(B, left, 2, right)
    c, sn = np.cos(theta / 2), np.sin(theta / 2)
    s0 = c * s[:, :, 0] - sn * s[:, :, 1]
    s1 = sn * s[:, :, 0] + c * s[:, :, 1]
    return np.stack([s0, s1], axis=2).reshape(B, DIM)


def _apply_cnot(S, q):
    B = S.shape[0]
    left, right = 1 << q, 1 << (N_QUBITS - q - 2)
    s = S.reshape(B, left, 2, 2, right)
    s = np.concatenate([s[:, :, :1], np.flip(s[:, :, 1:], axis=3)], axis=2)
    return s.reshape(B, DIM)


def _build_W(params):
    """Circuit applied to basis rows e_0..e_783 -> W[784, 1024], fp64."""
    w = np.pi * np.tanh(params.astype(np.float64))
    S = np.zeros((D_IN, DIM), dtype=np.float64)
    S[np.arange(D_IN), np.arange(D_IN)] = 1.0
    for l in range(params.shape[0]):
        for start in (0, 1):
            for i in range(start, N_QUBITS - 1, 2):
                S = _apply_cnot(S, i)
        for i in range(N_QUBITS):
            S = _apply_ry(S, w[l, i], i)
    return S


def _build_Z():
    z = np.arange(DIM)
    Z = np.zeros((DIM, ZCOLS), dtype=np.float32)
    for p in range(N_OUT):
        Z[:, p] = 1.0 - 2.0 * ((z >> (N_QUBITS - 1 - p)) & 1)
    Z[:, N_OUT] = 1.0
    # device layout [128, 8*16]: chunk z of rows z*128..z*128+128 at cols z*16..
    Zd = Z.reshape(N_ZCH, Z_CHUNK, ZCOLS).transpose(1, 0, 2).reshape(Z_CHUNK, -1)
    return np.ascontiguousarray(Zd)


def _round_f32r(a):
    """Round fp32 to the float32r encoding: e8m11 (RNE), i.e. IEEE fp32 with
    the low 12 mantissa bits cleared.  walrus's fp32_to_fp32r packs
    sign/exp/11-bit mantissa into the top 20 bits -- identical bit layout."""
    u = np.ascontiguousarray(a, dtype=np.float32).view(np.uint32)
    keep = u & np.uint32(0xFFFFF000)
    rem = u & np.uint32(0xFFF)
    inc = (rem > 0x800) | ((rem == 0x800) & (((u >> 12) & 1) == 1))
    out = keep + (inc.astype(np.uint32) << 12)
    return out.view(np.float32)


# ----------------------------------------------------------------------------
# Bass program (identical SPMD program on all 8 cores)
# ----------------------------------------------------------------------------

_NC_CACHE = {}
TRACE = False           # test harness can flip this for profiling
LAST_RESULTS = None


def _build_bass():
    from contextlib import ExitStack

    import concourse.tile as tile
    from concourse import bacc, mybir

    f32 = mybir.dt.float32
    f32r = mybir.dt.float32r

    nc = bacc.Bacc(
        "TRN2", target_bir_lowering=False, debug=False, num_devices=N_CORES
    )
    xt_d = nc.declare_dram_parameter("xt", [D_IN, B_CORE], f32r, isOutput=False)
    wt_d = nc.declare_dram_parameter("wt", [D_IN, DIM], f32r, isOutput=False)
    zt_d = nc.declare_dram_parameter(
        "zt", [Z_CHUNK, N_ZCH * ZCOLS], f32r, isOutput=False
    )
    out_d = nc.declare_dram_parameter("out", [N_OUT + 1, B_CORE], f32, isOutput=True)

    with ExitStack() as ctx:
        tc = ctx.enter_context(tile.TileContext(nc))
        wpool = ctx.enter_context(tc.tile_pool(name="w", bufs=1))
        zpool = ctx.enter_context(tc.tile_pool(name="z", bufs=1))
        xpool = ctx.enter_context(tc.tile_pool(name="x", bufs=1))
        sqpool = ctx.enter_context(tc.tile_pool(name="sq", bufs=1))
        opool = ctx.enter_context(tc.tile_pool(name="osb", bufs=1))
        # 7 py banks + 1 po bank = all 8 PSUM banks
        pypool = ctx.enter_context(tc.tile_pool(name="py", bufs=7, space="PSUM"))
        popool = ctx.enter_context(tc.tile_pool(name="po", bufs=1, space="PSUM"))

        # PE pre-warm: tiny dependency-free bf16 matmuls keep the tensor
        # engine busy during the DMA prefix so the HAM clock-gate releases
        # (1.2 -> 2.4 GHz) before real work arrives.
        bf16 = mybir.dt.bfloat16
        warm_in = opool.tile([1, 384], bf16, name="warm_in")
        nc.any.memset(warm_in[:], 0.0)
        warm_ps = popool.tile([128, 256], f32, name="warm_ps", tag="po")
        for i in range(28):
            nc.tensor.matmul(
                warm_ps[:],
                lhsT=warm_in[:, 0:128],
                rhs=warm_in[:, 128:384],
                start=True,
                stop=True,
                skip_group_check=True,
            )

        # All input DMAs on ONE HWDGE ring (sync), in consumption order:
        # (x_g0_j, w_j) pairs interleaved, then the sign matrix, then the
        # remaining x groups.  A single FIFO ring gives the fastest
        # per-DMA completion in issue order; a second ring would interleave
        # packets on the shared SDMA engines and delay pair landings.
        x_sb = [[None] * N_KCH for _ in range(N_GROUPS)]
        z_sb = zpool.tile([Z_CHUNK, N_ZCH * ZCOLS], f32r)
        w_sb = []
        for j in range(N_KCH):
            t = xpool.tile([K_CHUNK, GROUP], f32r, tag=f"x0_{j}")
            nc.sync.dma_start(t[:], xt_d[j * K_CHUNK:(j + 1) * K_CHUNK, 0:GROUP])
            x_sb[0][j] = t
            t = wpool.tile([K_CHUNK, DIM], f32r, tag=f"w{j}")
            nc.sync.dma_start(t[:], wt_d[j * K_CHUNK:(j + 1) * K_CHUNK, :])
            w_sb.append(t)
            if j == 0:
                # sign matrix (tiny, needed by the first mm2)
                nc.sync.dma_start(z_sb[:], zt_d[:, :])
        for g in range(1, N_GROUPS):
            for j in range(N_KCH):
                t = xpool.tile([K_CHUNK, GROUP], f32r, tag=f"x{g}_{j}")
                nc.sync.dma_start(
                    t[:],
                    xt_d[j * K_CHUNK:(j + 1) * K_CHUNK, g * GROUP:(g + 1) * GROUP],
                )
                x_sb[g][j] = t

        out_sb = opool.tile([N_OUT + 1, B_CORE], f32)

        for g in range(N_GROUPS):
            # j outer / z inner: each landed (x_j, w_j) pair unlocks 8 matmuls
            # (7 py banks; the 8th z waits for the first square to free a bank)
            pys = [
                pypool.tile([Z_CHUNK, GROUP], f32, tag="py", name=f"py_{g}_{z}")
                for z in range(N_ZCH)
            ]
            for j in range(N_KCH):
                for z in range(N_ZCH):
                    nc.tensor.matmul(
                        pys[z][:],
                        lhsT=w_sb[j][:, z * Z_CHUNK:(z + 1) * Z_CHUNK],
                        rhs=x_sb[g][j][:],
                        start=(j == 0),
                        stop=(j == N_KCH - 1),
                        skip_group_check=True,
                    )
            po = popool.tile([N_OUT + 1, GROUP], f32)
            for z in range(N_ZCH):
                sq = sqpool.tile([Z_CHUNK, GROUP], f32r, tag=f"sq{z}")
                nc.scalar.square(sq[:], pys[z][:])
                nc.tensor.matmul(
                    po[:],
                    lhsT=z_sb[:, z * ZCOLS: z * ZCOLS + N_OUT + 1],
                    rhs=sq[:],
                    start=(z == 0),
                    stop=(z == N_ZCH - 1),
                    skip_group_check=True,
                )
            nc.vector.tensor_copy(out_sb[:, g * GROUP:(g + 1) * GROUP], po[:])
            nc.sync.dma_start(
                out_d[:, g * GROUP:(g + 1) * GROUP],
                out_sb[:, g * GROUP:(g + 1) * GROUP],
            )

    nc.finalize()
    return nc


def _get_nc():
    if "nc" not in _NC_CACHE:
        _NC_CACHE["nc"] = _build_bass()
    return _NC_CACHE["nc"]


# ----------------------------------------------------------------------------
# Entry point
# ----------------------------------------------------------------------------

def kernel(input, params):
    global LAST_RESULTS
    from concourse.bass_utils import run_bass_kernel_spmd

    x = np.ascontiguousarray(np.asarray(input, dtype=np.float32))
    p = np.asarray(params, dtype=np.float32)

    W = _round_f32r(_build_W(p).astype(np.float32))            # [784, 1024]
    Z = _round_f32r(_build_Z())                                # [1024, 16]

    nc = _get_nc()
    in_maps = []
    for c in range(N_CORES):
        xt = _round_f32r(x[c * B_CORE:(c + 1) * B_CORE].T)           # [784, 2048]
        in_maps.append({"xt": xt, "wt": W, "zt": Z})

    res = run_bass_kernel_spmd(nc, in_maps, list(range(N_CORES)), trace=TRACE)
    LAST_RESULTS = res

    outs = []
    for c in range(N_CORES):
        o = res.results[c]["out"]                 # [11, 2048]
        outs.append((o[:N_OUT] / o[N_OUT:N_OUT + 1]).T)
    return np.ascontiguousarray(np.concatenate(outs, axis=0).astype(np.float32))



# revision 8
# speedup vs baseline: 1.2564x; 1.2564x over previous
"""Trainium2 kernel for nn_Circuit_41936060678727.

The reference is a 10-qubit real-amplitude circuit (CNOT ladders + RY
rotations) applied to an amplitude-embedded batch, measured with PauliZ on
each of the 10 wires.  Every gate is linear in the state, so the whole
8-layer circuit collapses to one fixed 784x1024 matrix W (orthonormal rows)
that depends only on `params`:

    out[b, p] = sum_z (x[b] @ W)[z]^2 * sign_p(z) / sum_z (x[b] @ W)[z]^2

The division makes the pipeline scale-invariant in y = x @ W, which lets the
matmul run in fp8 with generous global scales (SW on W, SX on centered x)
that keep everything out of e4m3's subnormal range.

Device math per core (2048 samples, data-parallel over 8 cores):
    mm1: y^T [1024, 2048] = Waug^T @ xaug         fp8 DoubleRow (0.5 cyc/row)
    sq    = y^2                                    scalar+vector engines, bf16
    mm2: o^T [11, 2048]  = Zsigns^T @ sq           bf16 (1 cyc/row)
Host: out = (o[:10] / o[10])^T, concat cores.

fp8 precision recovery (sim rel err ~1.3e-2 vs 2e-2 gate):
  - x is centered (x - 0.5) so its fp8 error halves; the constant shift is
    restored by 4 "bias rows" (ones on the x side, an fp8 split of
    0.5*colsum(W) on the W side) folded into the contraction for free.
  - 784 main rows pad to 5 DoubleRow chunks of 256 rows; the 496 spare
    slots carry residual-correction rows (W - fp8(W) paired with the same
    x values) that cancel most of the W quantization error.  Chunk 3 is
    the rows-0:256 residual at e5m2 (wide exponent range) and reuses the
    chunk-0 x tile already in SBUF.
"""

import numpy as np
import ml_dtypes

N_QUBITS = 10
DIM = 1 << N_QUBITS          # 1024
N_OUT = 10
D_IN = 784
B_TOTAL = 16384
N_CORES = 8
B_CORE = B_TOTAL // N_CORES  # 2048
GROUP = 512                  # batch columns per matmul (one PSUM bank, fp32)
N_GROUPS = B_CORE // GROUP   # 4
Z_CHUNK = 128
N_ZCH = DIM // Z_CHUNK       # 8
ZCOLS = 16                   # cols 0..9 = PauliZ signs, 10 = ones, 11..15 pad
NCH = 5                      # DoubleRow K-chunks of 256 rows (1280 slots)
SW = 64.0                    # global W scale (pulls W out of e4m3 subnormals)
SX = 4.0                     # global centered-x scale
N_WARM = 24                  # PE warm-up matmuls (clock ramp + DMA prefix)

E4 = ml_dtypes.float8_e4m3
E5 = ml_dtypes.float8_e5m2


# ----------------------------------------------------------------------------
# Host-side precompute: collapse the circuit to W = M[:784, :]
# ----------------------------------------------------------------------------

def _apply_ry(S, theta, q):
    B = S.shape[0]
    left, right = 1 << q, 1 << (N_QUBITS - q - 1)
    s = S.reshape(B, left, 2, right)
    c, sn = np.cos(theta / 2), np.sin(theta / 2)
    s0 = c * s[:, :, 0] - sn * s[:, :, 1]
    s1 = sn * s[:, :, 0] + c * s[:, :, 1]
    return np.stack([s0, s1], axis=2).reshape(B, DIM)


def _apply_cnot(S, q):
    B = S.shape[0]
    left, right = 1 << q, 1 << (N_QUBITS - q - 2)
    s = S.reshape(B, left, 2, 2, right)
    s = np.concatenate([s[:, :, :1], np.flip(s[:, :, 1:], axis=3)], axis=2)
    return s.reshape(B, DIM)


def _build_W(params):
    """Circuit applied to basis rows e_0..e_783 -> W[784, 1024], fp64."""
    w = np.pi * np.tanh(params.astype(np.float64))
    S = np.zeros((D_IN, DIM), dtype=np.float64)
    S[np.arange(D_IN), np.arange(D_IN)] = 1.0
    for l in range(params.shape[0]):
        for start in (0, 1):
            for i in range(start, N_QUBITS - 1, 2):
                S = _apply_cnot(S, i)
        for i in range(N_QUBITS):
            S = _apply_ry(S, w[l, i], i)
    return S


def _build_Z():
    z = np.arange(DIM)
    Z = np.zeros((DIM, ZCOLS), dtype=np.float32)
    for p in range(N_OUT):
        Z[:, p] = 1.0 - 2.0 * ((z >> (N_QUBITS - 1 - p)) & 1)
    Z[:, N_OUT] = 1.0
    # device layout [128, 8*16]: z-chunk c rows c*128..c*128+128 at cols c*16..
    Zd = Z.reshape(N_ZCH, Z_CHUNK, ZCOLS).transpose(1, 0, 2).reshape(Z_CHUNK, -1)
    return np.ascontiguousarray(Zd)


def _q(a, t):
    return np.asarray(a, np.float32).astype(t)


def _chunk_to_tile(A):
    """[256 aug rows, n] -> [128 partitions, 2 halves, n]; slot (p, i) holds
    aug row i*128 + p.  Must match between the W and x sides (it does)."""
    n = A.shape[1]
    return np.ascontiguousarray(A.reshape(2, 128, n).transpose(1, 0, 2))


def _build_weight_operands(params):
    """Returns (w4 [8, 128, 4, 256] e4m3, w5 [8, 128, 256] e5m2)."""
    W = _build_W(params)                     # fp64 [784, 1024]
    Ws = W * SW
    Wh = _q(Ws, E4)                          # main fp8 weights
    Wl = Ws - Wh.astype(np.float64)          # residual
    c_s = 0.5 * W.sum(axis=0) * SW * SX      # centering bias, scaled domain
    bias = []
    r = c_s.copy()
    b = _q(r / 2, E4); bias.append(b); r -= b.astype(np.float64)
    for _ in range(3):
        b = _q(r, E4); bias.append(b); r -= b.astype(np.float64)

    # e4m3 chunks in processing order [rows 0:256, 256:512, 512:768, mixed]
    che4 = [
        Wh[0:256], Wh[256:512], Wh[512:768],
        np.concatenate([
            _q(Ws[768:784], E4),             # main tail rows 768..783
            np.stack(bias, axis=0),          # 4 bias rows (x side = ones)
            _q(Wl[256:492], E4),             # residual rows 256..491
        ], axis=0),
    ]
    ch5 = _q(Wl[0:256], E5)                  # residual rows 0..255 at e5m2

    w4 = np.empty((N_ZCH, 128, 4, 2 * Z_CHUNK), dtype=E4)
    w5 = np.empty((N_ZCH, 128, 2 * Z_CHUNK), dtype=E5)
    for j, A in enumerate(che4):
        T = _chunk_to_tile(np.ascontiguousarray(A))      # [128, 2, 1024]
        for z in range(N_ZCH):
            blk = T[:, :, z * Z_CHUNK:(z + 1) * Z_CHUNK]  # [128, 2, 128]
            w4[z, :, j, :] = blk.reshape(128, 2 * Z_CHUNK)
    T = _chunk_to_tile(ch5)
    for z in range(N_ZCH):
        w5[z] = T[:, :, z * Z_CHUNK:(z + 1) * Z_CHUNK].reshape(128, 2 * Z_CHUNK)
    return w4, w5


def _build_x_operand(x_core):
    """x [2048, 784] f32 -> xt [4, 128, 2, 2048] e4m3 (aug chunks 0-2 main,
    chunk 3 = [tail rows 768:784, ones x4, rows 256:492])."""
    xs = (x_core.astype(np.float64) - 0.5) * SX
    xh = _q(xs, E4)                          # [2048, 784]
    xT = np.ascontiguousarray(xh.T)          # [784, 2048]
    ones = np.ones((4, B_CORE), dtype=E4)
    chunks = [
        xT[0:256], xT[256:512], xT[512:768],
        np.concatenate([xT[768:784], ones, xT[256:492]], axis=0),
    ]
    xt = np.empty((4, 128, 2, B_CORE), dtype=E4)
    for c, A in enumerate(chunks):
        xt[c] = _chunk_to_tile(np.ascontiguousarray(A))
    return xt


def _round_f32r(a):
    """fp32 -> float32r encoding (e8m11, RNE): low 12 mantissa bits cleared."""
    u = np.ascontiguousarray(a, dtype=np.float32).view(np.uint32)
    keep = u & np.uint32(0xFFFFF000)
    rem = u & np.uint32(0xFFF)
    inc = (rem > 0x800) | ((rem == 0x800) & (((u >> 12) & 1) == 1))
    out = keep + (inc.astype(np.uint32) << 12)
    return out.view(np.float32)


# ----------------------------------------------------------------------------
# Bass program (identical SPMD program on all 8 cores)
# ----------------------------------------------------------------------------

_NC_CACHE = {}
TRACE = False           # test harness can flip this for profiling
LAST_RESULTS = None


def _build_bass():
    from contextlib import ExitStack

    import concourse.tile as tile
    from concourse import bacc, mybir

    f32 = mybir.dt.float32
    f32r = mybir.dt.float32r
    f8e4 = mybir.dt.float8e4
    f8e5 = mybir.dt.float8e5
    bf16 = mybir.dt.bfloat16
    DR = mybir.MatmulPerfMode.DoubleRow
    MULT = mybir.AluOpType.mult

    nc = bacc.Bacc(
        "TRN2", target_bir_lowering=False, debug=False, num_devices=N_CORES
    )
    xt_d = nc.declare_dram_parameter("xt", [4, 128, 2, B_CORE], f8e4, isOutput=False)
    w4_d = nc.declare_dram_parameter("w4", [N_ZCH, 128, 4, 256], f8e4, isOutput=False)
    w5_d = nc.declare_dram_parameter("w5", [N_ZCH, 128, 256], f8e5, isOutput=False)
    zt_d = nc.declare_dram_parameter("zt", [Z_CHUNK, N_ZCH * ZCOLS], f32r, isOutput=False)
    out_d = nc.declare_dram_parameter("out", [N_OUT + 1, B_CORE], f32, isOutput=True)

    HALF = B_CORE // 2           # 1024 columns per phase

    with ExitStack() as ctx:
        tc = ctx.enter_context(tile.TileContext(nc))
        wpool = ctx.enter_context(tc.tile_pool(name="w", bufs=1))
        xpool = ctx.enter_context(tc.tile_pool(name="x", bufs=1))
        zpool = ctx.enter_context(tc.tile_pool(name="z", bufs=1))
        sqpool = ctx.enter_context(tc.tile_pool(name="sq", bufs=2))
        opool = ctx.enter_context(tc.tile_pool(name="osb", bufs=1))
        pypool = ctx.enter_context(tc.tile_pool(name="py", bufs=2, space="PSUM"))
        popool = ctx.enter_context(tc.tile_pool(name="po", bufs=1, space="PSUM"))

        # PE pre-warm: dependency-free bf16 matmuls keep the tensor engine
        # busy during the DMA prefix so the HAM clock-gate releases
        # (1.2 -> 2.4 GHz) before real work arrives.
        warm_in = opool.tile([1, 384], bf16, name="warm_in")
        nc.any.memset(warm_in[:], 0.0)
        warm_ps = popool.tile([128, 256], f32, name="warm_ps", tag="warm")
        for _ in range(N_WARM):
            nc.tensor.matmul(
                warm_ps[:],
                lhsT=warm_in[:, 0:128],
                rhs=warm_in[:, 128:384],
                start=True,
                stop=True,
                skip_group_check=True,
            )

        # Input DMAs, split over two rings (sync + gpsimd) so the phase-0
        # critical set (signs, z0 weights, first-half x) streams in parallel.
        z_sb = zpool.tile([Z_CHUNK, N_ZCH * ZCOLS], f32r)
        nc.sync.dma_start(z_sb[:], zt_d[:, :])
        w4_sb, w5_sb = [], []
        for z in range(N_ZCH):
            t4 = wpool.tile([128, 4, 2, Z_CHUNK], f8e4, tag=f"w4_{z}", name=f"w4_{z}")
            nc.sync.dma_start(t4[:], w4_d[z, :, :, :])
            t5 = wpool.tile([128, 2, Z_CHUNK], f8e5, tag=f"w5_{z}", name=f"w5_{z}")
            nc.sync.dma_start(t5[:], w5_d[z, :, :])
            w4_sb.append(t4)
            w5_sb.append(t5)
            if z == 0:
                x_sb = [[None, None] for _ in range(4)]
                for c in (0, 1):
                    t = xpool.tile([128, 2, HALF], f8e4, tag=f"x{c}h0", name=f"x{c}h0")
                    nc.sync.dma_start(t[:], xt_d[c, :, :, 0:HALF])
                    x_sb[c][0] = t
        for c in (2, 3):
            t = xpool.tile([128, 2, HALF], f8e4, tag=f"x{c}h0", name=f"x{c}h0")
            nc.gpsimd.dma_start(t[:], xt_d[c, :, :, 0:HALF])
            x_sb[c][0] = t
        for c in (0, 1):
            t = xpool.tile([128, 2, HALF], f8e4, tag=f"x{c}h1", name=f"x{c}h1")
            nc.sync.dma_start(t[:], xt_d[c, :, :, HALF:B_CORE])
            x_sb[c][1] = t
        for c in (2, 3):
            t = xpool.tile([128, 2, HALF], f8e4, tag=f"x{c}h1", name=f"x{c}h1")
            nc.gpsimd.dma_start(t[:], xt_d[c, :, :, HALF:B_CORE])
            x_sb[c][1] = t

        out_sb = opool.tile([N_OUT + 1, B_CORE], f32)

        def emit_mm2(pos, sqs, z):
            for gl in (0, 1):
                nc.tensor.matmul(
                    pos[gl][:],
                    lhsT=z_sb[:, z * ZCOLS:z * ZCOLS + N_OUT + 1],
                    rhs=sqs[gl][:],
                    start=(z == 0),
                    stop=(z == N_ZCH - 1),
                    skip_group_check=True,
                )

        # chunk processing order: the e5m2 residual chunk reuses the chunk-0
        # x tile, so it runs second (its x has already landed).
        for h in range(2):
            pos = [
                popool.tile([N_OUT + 1, GROUP], f32, tag=f"po{gl}", name=f"po_{h}_{gl}")
                for gl in (0, 1)
            ]
            prev = None
            for z in range(N_ZCH):
                seq = [
                    (w4_sb[z][:, 0], 0),
                    (w5_sb[z][:], 0),
                    (w4_sb[z][:, 1], 1),
                    (w4_sb[z][:, 2], 2),
                    (w4_sb[z][:, 3], 3),
                ]
                pys = [
                    pypool.tile([Z_CHUNK, GROUP], f32, tag=f"py{gl}",
                                name=f"py_{h}_{z}_{gl}")
                    for gl in (0, 1)
                ]
                for ci, (lhs, xc) in enumerate(seq):
                    for gl in (0, 1):
                        nc.tensor.matmul(
                            pys[gl][:],
                            lhsT=lhs,
                            rhs=x_sb[xc][h][:, :, gl * GROUP:(gl + 1) * GROUP],
                            start=(ci == 0),
                            stop=(ci == NCH - 1),
                            perf_mode=DR,
                            skip_group_check=True,
                        )
                    if ci == NCH - 2 and prev is not None:
                        # slot the previous z's sign-contraction into the
                        # middle of this z's mm1 so it never stalls on the
                        # squares' latency
                        emit_mm2(pos, prev, z - 1)
                # squares: scalar lane direct; DVE lane stages bf16 to SBUF
                # (walrus forbids two PSUM reads in one TensorTensor), then
                # squares in the 2-byte 2x DVE mode.
                sqs = []
                for gl in (0, 1):
                    sq = sqpool.tile(
                        [Z_CHUNK, GROUP], f32r, tag=f"sq{gl}", name=f"sq_{h}_{z}_{gl}"
                    )
                    if gl == 0:
                        nc.scalar.square(sq[:], pys[0][:])
                    else:
                        yc = sqpool.tile(
                            [Z_CHUNK, GROUP], f32r, tag="yc", name=f"yc_{h}_{z}"
                        )
                        nc.vector.tensor_copy(yc[:], pys[1][:])
                        nc.vector.tensor_tensor(sq[:], yc[:], yc[:], MULT)
                    sqs.append(sq)
                prev = sqs
            emit_mm2(pos, prev, N_ZCH - 1)
            off = h * HALF
            nc.scalar.copy(out_sb[:, off:off + GROUP], pos[0][:])
            nc.vector.tensor_copy(
                out_sb[:, off + GROUP:off + HALF], pos[1][:]
            )
            nc.sync.dma_start(out_d[:, off:off + HALF], out_sb[:, off:off + HALF])

    nc.finalize()
    return nc


def _get_nc():
    if "nc" not in _NC_CACHE:
        _NC_CACHE["nc"] = _build_bass()
    return _NC_CACHE["nc"]


# ----------------------------------------------------------------------------
# Entry point
# ----------------------------------------------------------------------------

def kernel(input, params):
    global LAST_RESULTS
    from concourse.bass_utils import run_bass_kernel_spmd

    x = np.ascontiguousarray(np.asarray(input, dtype=np.float32))
    p = np.asarray(params, dtype=np.float32)

    w4, w5 = _build_weight_operands(p)
    Z = _round_f32r(_build_Z())

    nc = _get_nc()
    in_maps = []
    for c in range(N_CORES):
        xt = _build_x_operand(x[c * B_CORE:(c + 1) * B_CORE])
        in_maps.append({"xt": xt, "w4": w4, "w5": w5, "zt": Z})

    res = run_bass_kernel_spmd(nc, in_maps, list(range(N_CORES)), trace=TRACE)
    LAST_RESULTS = res

    outs = []
    for c in range(N_CORES):
        o = res.results[c]["out"]                 # [11, 2048]
        outs.append((o[:N_OUT] / o[N_OUT:N_OUT + 1]).T)
    return np.ascontiguousarray(np.concatenate(outs, axis=0).astype(np.float32))


# revision 9
# speedup vs baseline: 1.3676x; 1.0885x over previous
"""Trainium2 kernel for nn_Circuit_41936060678727.

The reference is a 10-qubit real-amplitude circuit (CNOT ladders + RY
rotations) applied to an amplitude-embedded batch, measured with PauliZ on
each of the 10 wires.  Every gate is linear in the state, so the whole
8-layer circuit collapses to one fixed 784x1024 matrix W (orthonormal rows)
that depends only on `params`:

    out[b, p] = sum_z (x[b] @ W)[z]^2 * sign_p(z) / sum_z (x[b] @ W)[z]^2

The division makes the pipeline scale-invariant in y = x @ W, which lets the
matmul run in fp8 with generous global scales (SW on W, SX on centered x)
that keep everything out of e4m3's subnormal range.

Device math per core (2048 samples, data-parallel over 8 cores):
    mm1: y^T [1024, 2048] = Waug^T @ xaug         fp8 DoubleRow (0.5 cyc/row)
    sq    = y^2                                    scalar+vector engines, bf16
    mm2: o^T [11, 2048]  = Zsigns^T @ sq           bf16 (1 cyc/row)
Host: out = (o[:10] / o[10])^T, concat cores.

fp8 precision recovery (sim rel err ~1.3e-2 vs 2e-2 gate):
  - x is centered (x - 0.5) so its fp8 error halves; the constant shift is
    restored by 4 "bias rows" (ones on the x side, an fp8 split of
    0.5*colsum(W) on the W side) folded into the contraction for free.
  - 784 main rows pad to 5 DoubleRow chunks of 256 rows; the 496 spare
    slots carry residual-correction rows (W - fp8(W) paired with the same
    x values) that cancel most of the W quantization error.  Chunk 3 is
    the rows-0:256 residual at e5m2 (wide exponent range) and reuses the
    chunk-0 x tile already in SBUF.
"""

import numpy as np
import ml_dtypes

N_QUBITS = 10
DIM = 1 << N_QUBITS          # 1024
N_OUT = 10
D_IN = 784
B_TOTAL = 16384
N_CORES = 8
B_CORE = B_TOTAL // N_CORES  # 2048
GROUP = 512                  # batch columns per matmul (one PSUM bank, fp32)
N_GROUPS = B_CORE // GROUP   # 4
Z_CHUNK = 128
N_ZCH = DIM // Z_CHUNK       # 8
ZCOLS = 16                   # cols 0..9 = PauliZ signs, 10 = ones, 11..15 pad
NCH = 4                      # DoubleRow K-chunks of 256 rows (1024 slots)
SW = 64.0                    # global W scale (pulls W out of e4m3 subnormals)
SX = 4.0                     # global centered-x scale
N_WARM = 40                  # PE warm-up matmuls (clock ramp + DMA prefix)

E4 = ml_dtypes.float8_e4m3
E5 = ml_dtypes.float8_e5m2


# ----------------------------------------------------------------------------
# Host-side precompute: collapse the circuit to W = M[:784, :]
# ----------------------------------------------------------------------------

def _apply_ry(S, theta, q):
    B = S.shape[0]
    left, right = 1 << q, 1 << (N_QUBITS - q - 1)
    s = S.reshape(B, left, 2, right)
    c, sn = np.cos(theta / 2), np.sin(theta / 2)
    s0 = c * s[:, :, 0] - sn * s[:, :, 1]
    s1 = sn * s[:, :, 0] + c * s[:, :, 1]
    return np.stack([s0, s1], axis=2).reshape(B, DIM)


def _apply_cnot(S, q):
    B = S.shape[0]
    left, right = 1 << q, 1 << (N_QUBITS - q - 2)
    s = S.reshape(B, left, 2, 2, right)
    s = np.concatenate([s[:, :, :1], np.flip(s[:, :, 1:], axis=3)], axis=2)
    return s.reshape(B, DIM)


def _build_W(params):
    """Circuit applied to basis rows e_0..e_783 -> W[784, 1024], fp64."""
    w = np.pi * np.tanh(params.astype(np.float64))
    S = np.zeros((D_IN, DIM), dtype=np.float64)
    S[np.arange(D_IN), np.arange(D_IN)] = 1.0
    for l in range(params.shape[0]):
        for start in (0, 1):
            for i in range(start, N_QUBITS - 1, 2):
                S = _apply_cnot(S, i)
        for i in range(N_QUBITS):
            S = _apply_ry(S, w[l, i], i)
    return S


def _build_Z():
    z = np.arange(DIM)
    Z = np.zeros((DIM, ZCOLS), dtype=np.float32)
    for p in range(N_OUT):
        Z[:, p] = 1.0 - 2.0 * ((z >> (N_QUBITS - 1 - p)) & 1)
    Z[:, N_OUT] = 1.0
    # device layout [128, 8*16]: z-chunk c rows c*128..c*128+128 at cols c*16..
    Zd = Z.reshape(N_ZCH, Z_CHUNK, ZCOLS).transpose(1, 0, 2).reshape(Z_CHUNK, -1)
    return np.ascontiguousarray(Zd)


def _q(a, t):
    return np.asarray(a, np.float32).astype(t)


def _chunk_to_tile(A):
    """[256 aug rows, n] -> [128 partitions, 2 halves, n]; slot (p, i) holds
    aug row i*128 + p.  Must match between the W and x sides (it does)."""
    n = A.shape[1]
    return np.ascontiguousarray(A.reshape(2, 128, n).transpose(1, 0, 2))


def _build_weight_operands(params):
    """Returns w4 [8, 128, 4, 256] e4m3."""
    W = _build_W(params)                     # fp64 [784, 1024]
    Ws = W * SW
    Wh = _q(Ws, E4)                          # main fp8 weights
    Wl = Ws - Wh.astype(np.float64)          # residual
    c_s = 0.5 * W.sum(axis=0) * SW * SX      # centering bias, scaled domain
    bias = []
    r = c_s.copy()
    b = _q(r / 2, E4); bias.append(b); r -= b.astype(np.float64)
    for _ in range(3):
        b = _q(r, E4); bias.append(b); r -= b.astype(np.float64)

    # e4m3 chunks in processing order [rows 0:256, 256:512, 512:768, mixed]
    che4 = [
        Wh[0:256], Wh[256:512], Wh[512:768],
        np.concatenate([
            _q(Ws[768:784], E4),             # main tail rows 768..783
            np.stack(bias, axis=0),          # 4 bias rows (x side = ones)
            _q(Wl[0:236], E4),               # residual rows 0..235
        ], axis=0),
    ]

    w4 = np.empty((N_ZCH, 128, 4, 2 * Z_CHUNK), dtype=E4)
    for j, A in enumerate(che4):
        T = _chunk_to_tile(np.ascontiguousarray(A))      # [128, 2, 1024]
        for z in range(N_ZCH):
            blk = T[:, :, z * Z_CHUNK:(z + 1) * Z_CHUNK]  # [128, 2, 128]
            w4[z, :, j, :] = blk.reshape(128, 2 * Z_CHUNK)
    return w4


def _build_x_operand(x_core):
    """x [2048, 784] f32 -> xt [4, 128, 2, 2048] e4m3 (aug chunks 0-2 main,
    chunk 3 = [tail rows 768:784, ones x4, rows 256:492])."""
    xs = (x_core.astype(np.float64) - 0.5) * SX
    xh = _q(xs, E4)                          # [2048, 784]
    xT = np.ascontiguousarray(xh.T)          # [784, 2048]
    ones = np.ones((4, B_CORE), dtype=E4)
    chunks = [
        xT[0:256], xT[256:512], xT[512:768],
        np.concatenate([xT[768:784], ones, xT[0:236]], axis=0),
    ]
    xt = np.empty((4, 128, 2, B_CORE), dtype=E4)
    for c, A in enumerate(chunks):
        xt[c] = _chunk_to_tile(np.ascontiguousarray(A))
    return xt


def _round_f32r(a):
    """fp32 -> float32r encoding (e8m11, RNE): low 12 mantissa bits cleared."""
    u = np.ascontiguousarray(a, dtype=np.float32).view(np.uint32)
    keep = u & np.uint32(0xFFFFF000)
    rem = u & np.uint32(0xFFF)
    inc = (rem > 0x800) | ((rem == 0x800) & (((u >> 12) & 1) == 1))
    out = keep + (inc.astype(np.uint32) << 12)
    return out.view(np.float32)


# ----------------------------------------------------------------------------
# Bass program (identical SPMD program on all 8 cores)
# ----------------------------------------------------------------------------

_NC_CACHE = {}
TRACE = False           # test harness can flip this for profiling
LAST_RESULTS = None


def _build_bass():
    from contextlib import ExitStack

    import concourse.tile as tile
    from concourse import bacc, mybir

    f32 = mybir.dt.float32
    f32r = mybir.dt.float32r
    f8e4 = mybir.dt.float8e4
    f8e5 = mybir.dt.float8e5
    bf16 = mybir.dt.bfloat16
    DR = mybir.MatmulPerfMode.DoubleRow
    MULT = mybir.AluOpType.mult

    nc = bacc.Bacc(
        "TRN2", target_bir_lowering=False, debug=False, num_devices=N_CORES
    )
    xt_d = nc.declare_dram_parameter("xt", [4, 128, 2, B_CORE], f8e4, isOutput=False)
    w4_d = nc.declare_dram_parameter("w4", [N_ZCH, 128, 4, 256], f8e4, isOutput=False)
    zt_d = nc.declare_dram_parameter("zt", [Z_CHUNK, N_ZCH * ZCOLS], f32r, isOutput=False)
    out_d = nc.declare_dram_parameter("out", [N_OUT + 1, B_CORE], f32, isOutput=True)

    HALF = B_CORE // 2           # 1024 columns per phase

    with ExitStack() as ctx:
        tc = ctx.enter_context(tile.TileContext(nc))
        wpool = ctx.enter_context(tc.tile_pool(name="w", bufs=1))
        xpool = ctx.enter_context(tc.tile_pool(name="x", bufs=1))
        zpool = ctx.enter_context(tc.tile_pool(name="z", bufs=1))
        sqpool = ctx.enter_context(tc.tile_pool(name="sq", bufs=2))
        opool = ctx.enter_context(tc.tile_pool(name="osb", bufs=1))
        pypool = ctx.enter_context(tc.tile_pool(name="py", bufs=2, space="PSUM"))
        popool = ctx.enter_context(tc.tile_pool(name="po", bufs=1, space="PSUM"))

        # PE pre-warm: dependency-free bf16 matmuls keep the tensor engine
        # busy during the DMA prefix so the HAM clock-gate releases
        # (1.2 -> 2.4 GHz) before real work arrives.
        warm_in = opool.tile([1, 384], bf16, name="warm_in")
        nc.any.memset(warm_in[:], 0.0)
        warm_ps = popool.tile([128, 256], f32, name="warm_ps", tag="warm")
        for _ in range(N_WARM):
            nc.tensor.matmul(
                warm_ps[:],
                lhsT=warm_in[:, 0:128],
                rhs=warm_in[:, 128:384],
                start=True,
                stop=True,
                skip_group_check=True,
            )

        # Input DMAs, split over two rings (sync + gpsimd) so the phase-0
        # critical set (signs, z0 weights, first-half x) streams in parallel.
        z_sb = zpool.tile([Z_CHUNK, N_ZCH * ZCOLS], f32r)
        nc.sync.dma_start(z_sb[:], zt_d[:, :])
        w4_sb = []
        for z in range(N_ZCH):
            t4 = wpool.tile([128, 4, 2, Z_CHUNK], f8e4, tag=f"w4_{z}", name=f"w4_{z}")
            nc.sync.dma_start(t4[:], w4_d[z, :, :, :])
            w4_sb.append(t4)
            if z == 0:
                x_sb = [[None, None] for _ in range(4)]
                for c in (0, 1):
                    t = xpool.tile([128, 2, HALF], f8e4, tag=f"x{c}h0", name=f"x{c}h0")
                    nc.sync.dma_start(t[:], xt_d[c, :, :, 0:HALF])
                    x_sb[c][0] = t
        for c in (2, 3):
            t = xpool.tile([128, 2, HALF], f8e4, tag=f"x{c}h0", name=f"x{c}h0")
            nc.gpsimd.dma_start(t[:], xt_d[c, :, :, 0:HALF])
            x_sb[c][0] = t
        for c in (0, 1):
            t = xpool.tile([128, 2, HALF], f8e4, tag=f"x{c}h1", name=f"x{c}h1")
            nc.sync.dma_start(t[:], xt_d[c, :, :, HALF:B_CORE])
            x_sb[c][1] = t
        for c in (2, 3):
            t = xpool.tile([128, 2, HALF], f8e4, tag=f"x{c}h1", name=f"x{c}h1")
            nc.gpsimd.dma_start(t[:], xt_d[c, :, :, HALF:B_CORE])
            x_sb[c][1] = t

        out_sb = opool.tile([N_OUT + 1, B_CORE], f32)

        def emit_mm2(pos, sqs, z):
            for gl in (0, 1):
                nc.tensor.matmul(
                    pos[gl][:],
                    lhsT=z_sb[:, z * ZCOLS:z * ZCOLS + N_OUT + 1],
                    rhs=sqs[gl][:],
                    start=(z == 0),
                    stop=(z == N_ZCH - 1),
                    skip_group_check=True,
                )

        # chunk processing order: the e5m2 residual chunk reuses the chunk-0
        # x tile, so it runs second (its x has already landed).
        for h in range(2):
            pos = [
                popool.tile([N_OUT + 1, GROUP], f32, tag=f"po{gl}", name=f"po_{h}_{gl}")
                for gl in (0, 1)
            ]
            prev = None
            for z in range(N_ZCH):
                seq = [(w4_sb[z][:, j], j) for j in range(4)]
                pys = [
                    pypool.tile([Z_CHUNK, GROUP], f32, tag=f"py{gl}",
                                name=f"py_{h}_{z}_{gl}")
                    for gl in (0, 1)
                ]
                for ci, (lhs, xc) in enumerate(seq):
                    for gl in (0, 1):
                        nc.tensor.matmul(
                            pys[gl][:],
                            lhsT=lhs,
                            rhs=x_sb[xc][h][:, :, gl * GROUP:(gl + 1) * GROUP],
                            start=(ci == 0),
                            stop=(ci == NCH - 1),
                            perf_mode=DR,
                            skip_group_check=True,
                        )
                    if ci == NCH - 2 and prev is not None:
                        # slot the previous z's sign-contraction into the
                        # middle of this z's mm1 so it never stalls on the
                        # squares' latency
                        emit_mm2(pos, prev, z - 1)
                # squares: scalar lane direct; DVE lane stages bf16 to SBUF
                # (walrus forbids two PSUM reads in one TensorTensor), then
                # squares in the 2-byte 2x DVE mode.
                sqs = []
                for gl in (0, 1):
                    sq = sqpool.tile(
                        [Z_CHUNK, GROUP], f32r, tag=f"sq{gl}", name=f"sq_{h}_{z}_{gl}"
                    )
                    if gl == 0:
                        nc.scalar.square(sq[:], pys[0][:])
                    else:
                        yc = sqpool.tile(
                            [Z_CHUNK, GROUP], f32r, tag="yc", name=f"yc_{h}_{z}"
                        )
                        nc.vector.tensor_copy(yc[:], pys[1][:])
                        nc.vector.tensor_tensor(sq[:], yc[:], yc[:], MULT)
                    sqs.append(sq)
                prev = sqs
            emit_mm2(pos, prev, N_ZCH - 1)
            off = h * HALF
            nc.scalar.copy(out_sb[:, off:off + GROUP], pos[0][:])
            nc.vector.tensor_copy(
                out_sb[:, off + GROUP:off + HALF], pos[1][:]
            )
            nc.scalar.dma_start(
                out_d[:, off:off + GROUP], out_sb[:, off:off + GROUP]
            )
            nc.gpsimd.dma_start(
                out_d[:, off + GROUP:off + HALF],
                out_sb[:, off + GROUP:off + HALF],
            )

    nc.finalize()
    return nc


def _get_nc():
    if "nc" not in _NC_CACHE:
        _NC_CACHE["nc"] = _build_bass()
    return _NC_CACHE["nc"]


# ----------------------------------------------------------------------------
# Entry point
# ----------------------------------------------------------------------------

def kernel(input, params):
    global LAST_RESULTS
    from concourse.bass_utils import run_bass_kernel_spmd

    x = np.ascontiguousarray(np.asarray(input, dtype=np.float32))
    p = np.asarray(params, dtype=np.float32)

    w4 = _build_weight_operands(p)
    Z = _round_f32r(_build_Z())

    nc = _get_nc()
    in_maps = []
    for c in range(N_CORES):
        xt = _build_x_operand(x[c * B_CORE:(c + 1) * B_CORE])
        in_maps.append({"xt": xt, "w4": w4, "zt": Z})

    res = run_bass_kernel_spmd(nc, in_maps, list(range(N_CORES)), trace=TRACE)
    LAST_RESULTS = res

    outs = []
    for c in range(N_CORES):
        o = res.results[c]["out"]                 # [11, 2048]
        outs.append((o[:N_OUT] / o[N_OUT:N_OUT + 1]).T)
    return np.ascontiguousarray(np.concatenate(outs, axis=0).astype(np.float32))


# revision 10
# speedup vs baseline: 1.3731x; 1.0040x over previous
"""Trainium2 kernel for nn_Circuit_41936060678727.

The reference is a 10-qubit real-amplitude circuit (CNOT ladders + RY
rotations) applied to an amplitude-embedded batch, measured with PauliZ on
each of the 10 wires.  Every gate is linear in the state, so the whole
8-layer circuit collapses to one fixed 784x1024 matrix W (orthonormal rows)
that depends only on `params`:

    out[b, p] = sum_z (x[b] @ W)[z]^2 * sign_p(z) / sum_z (x[b] @ W)[z]^2

The division makes the pipeline scale-invariant in y = x @ W, which lets the
matmul run in fp8 with generous global scales (SW on W, SX on centered x)
that keep everything out of e4m3's subnormal range.

Device math per core (2048 samples, data-parallel over 8 cores):
    mm1: y^T [1024, 2048] = Waug^T @ xaug         fp8 DoubleRow (0.5 cyc/row)
    sq    = y^2                                    scalar+vector engines, bf16
    mm2: o^T [11, 2048]  = Zsigns^T @ sq           bf16 (1 cyc/row)
Host: out = (o[:10] / o[10])^T, concat cores.

fp8 precision recovery (sim rel err ~1.3e-2 vs 2e-2 gate):
  - x is centered (x - 0.5) so its fp8 error halves; the constant shift is
    restored by 4 "bias rows" (ones on the x side, an fp8 split of
    0.5*colsum(W) on the W side) folded into the contraction for free.
  - 784 main rows pad to 5 DoubleRow chunks of 256 rows; the 496 spare
    slots carry residual-correction rows (W - fp8(W) paired with the same
    x values) that cancel most of the W quantization error.  Chunk 3 is
    the rows-0:256 residual at e5m2 (wide exponent range) and reuses the
    chunk-0 x tile already in SBUF.
"""

import numpy as np
import ml_dtypes

N_QUBITS = 10
DIM = 1 << N_QUBITS          # 1024
N_OUT = 10
D_IN = 784
B_TOTAL = 16384
N_CORES = 8
B_CORE = B_TOTAL // N_CORES  # 2048
GROUP = 512                  # batch columns per matmul (one PSUM bank, fp32)
N_GROUPS = B_CORE // GROUP   # 4
Z_CHUNK = 128
N_ZCH = DIM // Z_CHUNK       # 8
ZCOLS = 16                   # cols 0..9 = PauliZ signs, 10 = ones, 11..15 pad
NCH = 4                      # DoubleRow K-chunks of 256 rows (1024 slots)
SW = 64.0                    # global W scale (pulls W out of e4m3 subnormals)
SX = 4.0                     # global centered-x scale
N_WARM = 36                  # PE warm-up matmuls (clock ramp + DMA prefix)

E4 = ml_dtypes.float8_e4m3
E5 = ml_dtypes.float8_e5m2


# ----------------------------------------------------------------------------
# Host-side precompute: collapse the circuit to W = M[:784, :]
# ----------------------------------------------------------------------------

def _apply_ry(S, theta, q):
    B = S.shape[0]
    left, right = 1 << q, 1 << (N_QUBITS - q - 1)
    s = S.reshape(B, left, 2, right)
    c, sn = np.cos(theta / 2), np.sin(theta / 2)
    s0 = c * s[:, :, 0] - sn * s[:, :, 1]
    s1 = sn * s[:, :, 0] + c * s[:, :, 1]
    return np.stack([s0, s1], axis=2).reshape(B, DIM)


def _apply_cnot(S, q):
    B = S.shape[0]
    left, right = 1 << q, 1 << (N_QUBITS - q - 2)
    s = S.reshape(B, left, 2, 2, right)
    s = np.concatenate([s[:, :, :1], np.flip(s[:, :, 1:], axis=3)], axis=2)
    return s.reshape(B, DIM)


def _build_W(params):
    """Circuit applied to basis rows e_0..e_783 -> W[784, 1024], fp64."""
    w = np.pi * np.tanh(params.astype(np.float64))
    S = np.zeros((D_IN, DIM), dtype=np.float64)
    S[np.arange(D_IN), np.arange(D_IN)] = 1.0
    for l in range(params.shape[0]):
        for start in (0, 1):
            for i in range(start, N_QUBITS - 1, 2):
                S = _apply_cnot(S, i)
        for i in range(N_QUBITS):
            S = _apply_ry(S, w[l, i], i)
    return S


def _build_Z():
    z = np.arange(DIM)
    Z = np.zeros((DIM, ZCOLS), dtype=np.float32)
    for p in range(N_OUT):
        Z[:, p] = 1.0 - 2.0 * ((z >> (N_QUBITS - 1 - p)) & 1)
    Z[:, N_OUT] = 1.0
    # device layout [128, 8*16]: z-chunk c rows c*128..c*128+128 at cols c*16..
    Zd = Z.reshape(N_ZCH, Z_CHUNK, ZCOLS).transpose(1, 0, 2).reshape(Z_CHUNK, -1)
    return np.ascontiguousarray(Zd)


def _q(a, t):
    return np.asarray(a, np.float32).astype(t)


def _chunk_to_tile(A):
    """[256 aug rows, n] -> [128 partitions, 2 halves, n]; slot (p, i) holds
    aug row i*128 + p.  Must match between the W and x sides (it does)."""
    n = A.shape[1]
    return np.ascontiguousarray(A.reshape(2, 128, n).transpose(1, 0, 2))


def _build_weight_operands(params):
    """Returns w4 [8, 128, 4, 256] e4m3."""
    W = _build_W(params)                     # fp64 [784, 1024]
    Ws = W * SW
    Wh = _q(Ws, E4)                          # main fp8 weights
    Wl = Ws - Wh.astype(np.float64)          # residual
    c_s = 0.5 * W.sum(axis=0) * SW * SX      # centering bias, scaled domain
    bias = []
    r = c_s.copy()
    b = _q(r / 2, E4); bias.append(b); r -= b.astype(np.float64)
    for _ in range(3):
        b = _q(r, E4); bias.append(b); r -= b.astype(np.float64)

    # e4m3 chunks in processing order [rows 0:256, 256:512, 512:768, mixed]
    che4 = [
        Wh[0:256], Wh[256:512], Wh[512:768],
        np.concatenate([
            _q(Ws[768:784], E4),             # main tail rows 768..783
            np.stack(bias, axis=0),          # 4 bias rows (x side = ones)
            _q(Wl[0:236], E4),               # residual rows 0..235
        ], axis=0),
    ]

    w4 = np.empty((N_ZCH, 128, 4, 2 * Z_CHUNK), dtype=E4)
    for j, A in enumerate(che4):
        T = _chunk_to_tile(np.ascontiguousarray(A))      # [128, 2, 1024]
        for z in range(N_ZCH):
            blk = T[:, :, z * Z_CHUNK:(z + 1) * Z_CHUNK]  # [128, 2, 128]
            w4[z, :, j, :] = blk.reshape(128, 2 * Z_CHUNK)
    return w4


def _build_x_operand(x_core):
    """x [2048, 784] f32 -> xt [4, 128, 2, 2048] e4m3 (aug chunks 0-2 main,
    chunk 3 = [tail rows 768:784, ones x4, rows 256:492])."""
    xs = (x_core.astype(np.float64) - 0.5) * SX
    xh = _q(xs, E4)                          # [2048, 784]
    xT = np.ascontiguousarray(xh.T)          # [784, 2048]
    ones = np.ones((4, B_CORE), dtype=E4)
    chunks = [
        xT[0:256], xT[256:512], xT[512:768],
        np.concatenate([xT[768:784], ones, xT[0:236]], axis=0),
    ]
    xt = np.empty((4, 128, 2, B_CORE), dtype=E4)
    for c, A in enumerate(chunks):
        xt[c] = _chunk_to_tile(np.ascontiguousarray(A))
    return xt


def _round_f32r(a):
    """fp32 -> float32r encoding (e8m11, RNE): low 12 mantissa bits cleared."""
    u = np.ascontiguousarray(a, dtype=np.float32).view(np.uint32)
    keep = u & np.uint32(0xFFFFF000)
    rem = u & np.uint32(0xFFF)
    inc = (rem > 0x800) | ((rem == 0x800) & (((u >> 12) & 1) == 1))
    out = keep + (inc.astype(np.uint32) << 12)
    return out.view(np.float32)


# ----------------------------------------------------------------------------
# Bass program (identical SPMD program on all 8 cores)
# ----------------------------------------------------------------------------

_NC_CACHE = {}
TRACE = False           # test harness can flip this for profiling
LAST_RESULTS = None


def _build_bass():
    from contextlib import ExitStack

    import concourse.tile as tile
    from concourse import bacc, mybir

    f32 = mybir.dt.float32
    f32r = mybir.dt.float32r
    f8e4 = mybir.dt.float8e4
    f8e5 = mybir.dt.float8e5
    bf16 = mybir.dt.bfloat16
    DR = mybir.MatmulPerfMode.DoubleRow
    MULT = mybir.AluOpType.mult

    nc = bacc.Bacc(
        "TRN2", target_bir_lowering=False, debug=False, num_devices=N_CORES
    )
    xt_d = nc.declare_dram_parameter("xt", [4, 128, 2, B_CORE], f8e4, isOutput=False)
    w4_d = nc.declare_dram_parameter("w4", [N_ZCH, 128, 4, 256], f8e4, isOutput=False)
    zt_d = nc.declare_dram_parameter("zt", [Z_CHUNK, N_ZCH * ZCOLS], f32r, isOutput=False)
    out_d = nc.declare_dram_parameter("out", [N_OUT + 1, B_CORE], f32, isOutput=True)

    HALF = B_CORE // 2           # 1024 columns per phase

    with ExitStack() as ctx:
        tc = ctx.enter_context(tile.TileContext(nc))
        wpool = ctx.enter_context(tc.tile_pool(name="w", bufs=1))
        xpool = ctx.enter_context(tc.tile_pool(name="x", bufs=1))
        zpool = ctx.enter_context(tc.tile_pool(name="z", bufs=1))
        sqpool = ctx.enter_context(tc.tile_pool(name="sq", bufs=2))
        opool = ctx.enter_context(tc.tile_pool(name="osb", bufs=1))
        pypool = ctx.enter_context(tc.tile_pool(name="py", bufs=2, space="PSUM"))
        popool = ctx.enter_context(tc.tile_pool(name="po", bufs=1, space="PSUM"))

        # PE pre-warm: dependency-free bf16 matmuls keep the tensor engine
        # busy during the DMA prefix so the HAM clock-gate releases
        # (1.2 -> 2.4 GHz) before real work arrives.
        warm_in = opool.tile([128, 384], bf16, name="warm_in")
        nc.any.memset(warm_in[:], 1.0)
        warm_ps = popool.tile([128, 256], f32, name="warm_ps", tag="warm")
        for _ in range(N_WARM):
            nc.tensor.matmul(
                warm_ps[:],
                lhsT=warm_in[:, 0:128],
                rhs=warm_in[:, 128:384],
                start=True,
                stop=True,
                skip_group_check=True,
            )

        # Input DMAs, split over two rings (sync + gpsimd) so the phase-0
        # critical set (signs, z0 weights, first-half x) streams in parallel.
        z_sb = zpool.tile([Z_CHUNK, N_ZCH * ZCOLS], f32r)
        nc.sync.dma_start(z_sb[:], zt_d[:, :])
        w4_sb = []
        for z in range(N_ZCH):
            t4 = wpool.tile([128, 4, 2, Z_CHUNK], f8e4, tag=f"w4_{z}", name=f"w4_{z}")
            nc.sync.dma_start(t4[:], w4_d[z, :, :, :])
            w4_sb.append(t4)
            if z == 0:
                x_sb = [[None, None] for _ in range(4)]
                for c in (0, 1):
                    t = xpool.tile([128, 2, HALF], f8e4, tag=f"x{c}h0", name=f"x{c}h0")
                    nc.sync.dma_start(t[:], xt_d[c, :, :, 0:HALF])
                    x_sb[c][0] = t
        for c in (2, 3):
            t = xpool.tile([128, 2, HALF], f8e4, tag=f"x{c}h0", name=f"x{c}h0")
            nc.gpsimd.dma_start(t[:], xt_d[c, :, :, 0:HALF])
            x_sb[c][0] = t
        for c in (0, 1):
            t = xpool.tile([128, 2, HALF], f8e4, tag=f"x{c}h1", name=f"x{c}h1")
            nc.sync.dma_start(t[:], xt_d[c, :, :, HALF:B_CORE])
            x_sb[c][1] = t
        for c in (2, 3):
            t = xpool.tile([128, 2, HALF], f8e4, tag=f"x{c}h1", name=f"x{c}h1")
            nc.gpsimd.dma_start(t[:], xt_d[c, :, :, HALF:B_CORE])
            x_sb[c][1] = t

        out_sb = opool.tile([N_OUT + 1, B_CORE], f32)

        def emit_mm2(pos, sqs, z):
            for gl in (0, 1):
                nc.tensor.matmul(
                    pos[gl][:],
                    lhsT=z_sb[:, z * ZCOLS:z * ZCOLS + N_OUT + 1],
                    rhs=sqs[gl][:],
                    start=(z == 0),
                    stop=(z == N_ZCH - 1),
                    skip_group_check=True,
                )

        # chunk processing order: the e5m2 residual chunk reuses the chunk-0
        # x tile, so it runs second (its x has already landed).
        for h in range(2):
            pos = [
                popool.tile([N_OUT + 1, GROUP], f32, tag=f"po{gl}", name=f"po_{h}_{gl}")
                for gl in (0, 1)
            ]
            prev = None
            for z in range(N_ZCH):
                seq = [(w4_sb[z][:, j], j) for j in range(4)]
                pys = [
                    pypool.tile([Z_CHUNK, GROUP], f32, tag=f"py{gl}",
                                name=f"py_{h}_{z}_{gl}")
                    for gl in (0, 1)
                ]
                for ci, (lhs, xc) in enumerate(seq):
                    for gl in (0, 1):
                        nc.tensor.matmul(
                            pys[gl][:],
                            lhsT=lhs,
                            rhs=x_sb[xc][h][:, :, gl * GROUP:(gl + 1) * GROUP],
                            start=(ci == 0),
                            stop=(ci == NCH - 1),
                            perf_mode=DR,
                            skip_group_check=True,
                        )
                    if ci == NCH - 2 and prev is not None:
                        # slot the previous z's sign-contraction into the
                        # middle of this z's mm1 so it never stalls on the
                        # squares' latency
                        emit_mm2(pos, prev, z - 1)
                # squares: scalar lane direct; DVE lane stages bf16 to SBUF
                # (walrus forbids two PSUM reads in one TensorTensor), then
                # squares in the 2-byte 2x DVE mode.
                sqs = []
                for gl in (0, 1):
                    sq = sqpool.tile(
                        [Z_CHUNK, GROUP], f32r, tag=f"sq{gl}", name=f"sq_{h}_{z}_{gl}"
                    )
                    if gl == 0:
                        nc.scalar.square(sq[:], pys[0][:])
                    else:
                        yc = sqpool.tile(
                            [Z_CHUNK, GROUP], f32r, tag="yc", name=f"yc_{h}_{z}"
                        )
                        nc.vector.tensor_copy(yc[:], pys[1][:])
                        nc.vector.tensor_tensor(sq[:], yc[:], yc[:], MULT)
                    sqs.append(sq)
                prev = sqs
            emit_mm2(pos, prev, N_ZCH - 1)
            off = h * HALF
            nc.scalar.copy(out_sb[:, off:off + GROUP], pos[0][:])
            nc.vector.tensor_copy(
                out_sb[:, off + GROUP:off + HALF], pos[1][:]
            )
            nc.scalar.dma_start(
                out_d[:, off:off + GROUP], out_sb[:, off:off + GROUP]
            )
            nc.sync.dma_start(
                out_d[:, off + GROUP:off + HALF],
                out_sb[:, off + GROUP:off + HALF],
            )

    nc.finalize()
    return nc


def _get_nc():
    if "nc" not in _NC_CACHE:
        _NC_CACHE["nc"] = _build_bass()
    return _NC_CACHE["nc"]


# ----------------------------------------------------------------------------
# Entry point
# ----------------------------------------------------------------------------

def kernel(input, params):
    global LAST_RESULTS
    from concourse.bass_utils import run_bass_kernel_spmd

    x = np.ascontiguousarray(np.asarray(input, dtype=np.float32))
    p = np.asarray(params, dtype=np.float32)

    w4 = _build_weight_operands(p)
    Z = _round_f32r(_build_Z())

    nc = _get_nc()
    in_maps = []
    for c in range(N_CORES):
        xt = _build_x_operand(x[c * B_CORE:(c + 1) * B_CORE])
        in_maps.append({"xt": xt, "w4": w4, "zt": Z})

    res = run_bass_kernel_spmd(nc, in_maps, list(range(N_CORES)), trace=TRACE)
    LAST_RESULTS = res

    outs = []
    for c in range(N_CORES):
        o = res.results[c]["out"]                 # [11, 2048]
        outs.append((o[:N_OUT] / o[N_OUT:N_OUT + 1]).T)
    return np.ascontiguousarray(np.concatenate(outs, axis=0).astype(np.float32))


# revision 11
# speedup vs baseline: 1.4148x; 1.0304x over previous
"""Trainium2 kernel for nn_Circuit_41936060678727.

The reference is a 10-qubit real-amplitude circuit (CNOT ladders + RY
rotations) applied to an amplitude-embedded batch, measured with PauliZ on
each of the 10 wires.  Every gate is linear in the state, so the whole
8-layer circuit collapses to one fixed 784x1024 matrix W (orthonormal rows)
that depends only on `params`:

    out[b, p] = sum_z (x[b] @ W)[z]^2 * sign_p(z) / sum_z (x[b] @ W)[z]^2

The division makes the pipeline scale-invariant in y = x @ W, which lets the
matmul run in fp8 with generous global scales (SW on W, SX on centered x)
that keep everything out of e4m3's subnormal range.

Device math per core (2048 samples, data-parallel over 8 cores):
    mm1: y^T [1024, 2048] = Waug^T @ xaug         fp8 DoubleRow (0.5 cyc/row)
    sq    = y^2                                    scalar+vector engines, bf16
    mm2: o^T [11, 2048]  = Zsigns^T @ sq           bf16 (1 cyc/row)
Host: out = (o[:10] / o[10])^T, concat cores.

fp8 precision recovery (sim rel err ~1.3e-2 vs 2e-2 gate):
  - x is centered (x - 0.5) so its fp8 error halves; the constant shift is
    restored by 4 "bias rows" (ones on the x side, an fp8 split of
    0.5*colsum(W) on the W side) folded into the contraction for free.
  - 784 main rows pad to 5 DoubleRow chunks of 256 rows; the 496 spare
    slots carry residual-correction rows (W - fp8(W) paired with the same
    x values) that cancel most of the W quantization error.  Chunk 3 is
    the rows-0:256 residual at e5m2 (wide exponent range) and reuses the
    chunk-0 x tile already in SBUF.
"""

import numpy as np
import ml_dtypes

N_QUBITS = 10
DIM = 1 << N_QUBITS          # 1024
N_OUT = 10
D_IN = 784
B_TOTAL = 16384
N_CORES = 8
B_CORE = B_TOTAL // N_CORES  # 2048
GROUP = 512                  # batch columns per matmul (one PSUM bank, fp32)
N_GROUPS = B_CORE // GROUP   # 4
Z_CHUNK = 128
N_ZCH = DIM // Z_CHUNK       # 8
ZCOLS = 16                   # cols 0..9 = PauliZ signs, 10 = ones, 11..15 pad
NCH = 4                      # DoubleRow K-chunks of 256 rows (1024 slots)
SW = 64.0                    # global W scale (pulls W out of e4m3 subnormals)
SX = 4.0                     # global centered-x scale
N_WARM = 36                  # PE warm-up matmuls (clock ramp + DMA prefix)

E4 = ml_dtypes.float8_e4m3
E5 = ml_dtypes.float8_e5m2


# ----------------------------------------------------------------------------
# Host-side precompute: collapse the circuit to W = M[:784, :]
# ----------------------------------------------------------------------------

def _apply_ry(S, theta, q):
    B = S.shape[0]
    left, right = 1 << q, 1 << (N_QUBITS - q - 1)
    s = S.reshape(B, left, 2, right)
    c, sn = np.cos(theta / 2), np.sin(theta / 2)
    s0 = c * s[:, :, 0] - sn * s[:, :, 1]
    s1 = sn * s[:, :, 0] + c * s[:, :, 1]
    return np.stack([s0, s1], axis=2).reshape(B, DIM)


def _apply_cnot(S, q):
    B = S.shape[0]
    left, right = 1 << q, 1 << (N_QUBITS - q - 2)
    s = S.reshape(B, left, 2, 2, right)
    s = np.concatenate([s[:, :, :1], np.flip(s[:, :, 1:], axis=3)], axis=2)
    return s.reshape(B, DIM)


def _build_W(params):
    """Circuit applied to basis rows e_0..e_783 -> W[784, 1024], fp64."""
    w = np.pi * np.tanh(params.astype(np.float64))
    S = np.zeros((D_IN, DIM), dtype=np.float64)
    S[np.arange(D_IN), np.arange(D_IN)] = 1.0
    for l in range(params.shape[0]):
        for start in (0, 1):
            for i in range(start, N_QUBITS - 1, 2):
                S = _apply_cnot(S, i)
        for i in range(N_QUBITS):
            S = _apply_ry(S, w[l, i], i)
    return S


def _build_Z():
    z = np.arange(DIM)
    Z = np.zeros((DIM, ZCOLS), dtype=np.float32)
    for p in range(N_OUT):
        Z[:, p] = 1.0 - 2.0 * ((z >> (N_QUBITS - 1 - p)) & 1)
    Z[:, N_OUT] = 1.0
    # device layout [128, 8*16]: z-chunk c rows c*128..c*128+128 at cols c*16..
    Zd = Z.reshape(N_ZCH, Z_CHUNK, ZCOLS).transpose(1, 0, 2).reshape(Z_CHUNK, -1)
    return np.ascontiguousarray(Zd)


def _q(a, t):
    return np.asarray(a, np.float32).astype(t)


def _chunk_to_tile(A):
    """[256 aug rows, n] -> [128 partitions, 2 halves, n]; slot (p, i) holds
    aug row i*128 + p.  Must match between the W and x sides (it does)."""
    n = A.shape[1]
    return np.ascontiguousarray(A.reshape(2, 128, n).transpose(1, 0, 2))


def _build_weight_operands(params):
    """Returns w4 [8, 128, 4, 256] e4m3."""
    W = _build_W(params)                     # fp64 [784, 1024]
    Ws = W * SW
    Wh = _q(Ws, E4)                          # main fp8 weights
    Wl = Ws - Wh.astype(np.float64)          # residual
    c_s = 0.5 * W.sum(axis=0) * SW * SX      # centering bias, scaled domain
    bias = []
    r = c_s.copy()
    b = _q(r / 2, E4); bias.append(b); r -= b.astype(np.float64)
    for _ in range(3):
        b = _q(r, E4); bias.append(b); r -= b.astype(np.float64)

    # e4m3 chunks in processing order [rows 0:256, 256:512, 512:768, mixed]
    che4 = [
        Wh[0:256], Wh[256:512], Wh[512:768],
        np.concatenate([
            _q(Ws[768:784], E4),             # main tail rows 768..783
            np.stack(bias, axis=0),          # 4 bias rows (x side = ones)
            _q(Wl[0:236], E4),               # residual rows 0..235
        ], axis=0),
    ]

    w4 = np.empty((N_ZCH, 128, 4, 2 * Z_CHUNK), dtype=E4)
    for j, A in enumerate(che4):
        T = _chunk_to_tile(np.ascontiguousarray(A))      # [128, 2, 1024]
        for z in range(N_ZCH):
            blk = T[:, :, z * Z_CHUNK:(z + 1) * Z_CHUNK]  # [128, 2, 128]
            w4[z, :, j, :] = blk.reshape(128, 2 * Z_CHUNK)
    return w4


def _build_x_operand(x_core):
    """x [2048, 784] f32 -> xt [4, 128, 2, 2048] e4m3 (aug chunks 0-2 main,
    chunk 3 = [tail rows 768:784, ones x4, rows 256:492])."""
    xs = (x_core.astype(np.float64) - 0.5) * SX
    xh = _q(xs, E4)                          # [2048, 784]
    xT = np.ascontiguousarray(xh.T)          # [784, 2048]
    ones = np.ones((4, B_CORE), dtype=E4)
    chunks = [
        xT[0:256], xT[256:512], xT[512:768],
        np.concatenate([xT[768:784], ones, xT[0:236]], axis=0),
    ]
    xt = np.empty((4, 128, 2, B_CORE), dtype=E4)
    for c, A in enumerate(chunks):
        xt[c] = _chunk_to_tile(np.ascontiguousarray(A))
    return xt


def _round_f32r(a):
    """fp32 -> float32r encoding (e8m11, RNE): low 12 mantissa bits cleared."""
    u = np.ascontiguousarray(a, dtype=np.float32).view(np.uint32)
    keep = u & np.uint32(0xFFFFF000)
    rem = u & np.uint32(0xFFF)
    inc = (rem > 0x800) | ((rem == 0x800) & (((u >> 12) & 1) == 1))
    out = keep + (inc.astype(np.uint32) << 12)
    return out.view(np.float32)


# ----------------------------------------------------------------------------
# Bass program (identical SPMD program on all 8 cores)
# ----------------------------------------------------------------------------

_NC_CACHE = {}
TRACE = False           # test harness can flip this for profiling
LAST_RESULTS = None


def _build_bass():
    from contextlib import ExitStack

    import concourse.tile as tile
    from concourse import bacc, mybir

    f32 = mybir.dt.float32
    f32r = mybir.dt.float32r
    f8e4 = mybir.dt.float8e4
    f8e5 = mybir.dt.float8e5
    bf16 = mybir.dt.bfloat16
    DR = mybir.MatmulPerfMode.DoubleRow
    MULT = mybir.AluOpType.mult

    nc = bacc.Bacc(
        "TRN2", target_bir_lowering=False, debug=False, num_devices=N_CORES
    )
    xt_d = nc.declare_dram_parameter("xt", [4, 128, 2, B_CORE], f8e4, isOutput=False)
    w4_d = nc.declare_dram_parameter("w4", [N_ZCH, 128, 4, 256], f8e4, isOutput=False)
    zt_d = nc.declare_dram_parameter("zt", [Z_CHUNK, N_ZCH * ZCOLS], f32r, isOutput=False)
    out_d = nc.declare_dram_parameter("out", [N_OUT + 1, B_CORE], f32, isOutput=True)

    N_PH = 4                      # column phases of one 512-col group each

    with ExitStack() as ctx:
        tc = ctx.enter_context(tile.TileContext(nc))
        wpool = ctx.enter_context(tc.tile_pool(name="w", bufs=1))
        xpool = ctx.enter_context(tc.tile_pool(name="x", bufs=1))
        zpool = ctx.enter_context(tc.tile_pool(name="z", bufs=1))
        sqpool = ctx.enter_context(tc.tile_pool(name="sq", bufs=2))
        opool = ctx.enter_context(tc.tile_pool(name="osb", bufs=1))
        pypool = ctx.enter_context(tc.tile_pool(name="py", bufs=2, space="PSUM"))
        popool = ctx.enter_context(tc.tile_pool(name="po", bufs=1, space="PSUM"))

        # PE pre-warm: K=128 bf16 matmuls with the array fully lit -- the
        # HAM/DVFS clock ramps on real array activity (K=1 warms leave it at
        # 1.2 GHz), and the warm window also covers the first x slab's DMA.
        warm_in = opool.tile([128, 384], bf16, name="warm_in")
        nc.any.memset(warm_in[:], 1.0)
        warm_ps = popool.tile([128, 256], f32, name="warm_ps", tag="warm")
        for _ in range(N_WARM):
            nc.tensor.matmul(
                warm_ps[:],
                lhsT=warm_in[:, 0:128],
                rhs=warm_in[:, 128:384],
                start=True,
                stop=True,
                skip_group_check=True,
            )

        # Input DMAs, split over two rings (sync + gpsimd) so the phase-0
        # critical set (signs, z0 weights, first x quarter) streams first.
        z_sb = zpool.tile([Z_CHUNK, N_ZCH * ZCOLS], f32r)
        nc.sync.dma_start(z_sb[:], zt_d[:, :])
        w4_sb = [None] * N_ZCH
        x_sb = [[None] * N_PH for _ in range(4)]

        def load_w(z):
            t = wpool.tile([128, 4, 2, Z_CHUNK], f8e4, tag=f"w4_{z}", name=f"w4_{z}")
            nc.sync.dma_start(t[:], w4_d[z, :, :, :])
            w4_sb[z] = t

        def load_x(c, q, eng):
            t = xpool.tile([128, 2, GROUP], f8e4, tag=f"x{c}q{q}", name=f"x{c}q{q}")
            eng.dma_start(t[:], xt_d[c, :, :, q * GROUP:(q + 1) * GROUP])
            x_sb[c][q] = t

        load_w(0)
        for q in range(N_PH):
            for c in (0, 1):
                load_x(c, q, nc.sync)
            if q < 3:
                load_w(q + 1)
        for z in range(4, N_ZCH):
            load_w(z)
        for q in range(N_PH):
            for c in (2, 3):
                load_x(c, q, nc.gpsimd)

        out_sb = opool.tile([N_OUT + 1, B_CORE], f32)

        for h in range(N_PH):
            po = popool.tile(
                [N_OUT + 1, GROUP], f32, tag=f"po{h % 2}", name=f"po_{h}"
            )

            def emit_mm2(sq, z):
                nc.tensor.matmul(
                    po[:],
                    lhsT=z_sb[:, z * ZCOLS:z * ZCOLS + N_OUT + 1],
                    rhs=sq[:],
                    start=(z == 0),
                    stop=(z == N_ZCH - 1),
                    skip_group_check=True,
                )

            prev = None
            for z in range(N_ZCH):
                py = pypool.tile(
                    [Z_CHUNK, GROUP], f32, tag="py", name=f"py_{h}_{z}"
                )
                for ci in range(4):
                    nc.tensor.matmul(
                        py[:],
                        lhsT=w4_sb[z][:, ci],
                        rhs=x_sb[ci][h][:],
                        start=(ci == 0),
                        stop=(ci == NCH - 1),
                        perf_mode=DR,
                        skip_group_check=True,
                    )
                if prev is not None:
                    emit_mm2(prev, z - 1)
                # squares alternate engines; the DVE lane stages bf16 to SBUF
                # (walrus forbids two PSUM reads in one TensorTensor).
                sq = sqpool.tile(
                    [Z_CHUNK, GROUP], f32r, tag="sq", name=f"sq_{h}_{z}"
                )
                if z % 2 == 0:
                    nc.scalar.square(sq[:], py[:])
                else:
                    yc = sqpool.tile(
                        [Z_CHUNK, GROUP], f32r, tag="yc", name=f"yc_{h}_{z}"
                    )
                    nc.vector.tensor_copy(yc[:], py[:])
                    nc.vector.tensor_tensor(sq[:], yc[:], yc[:], MULT)
                prev = sq
            emit_mm2(prev, N_ZCH - 1)
            off = h * GROUP
            if h % 2 == 0:
                nc.scalar.copy(out_sb[:, off:off + GROUP], po[:])
                nc.scalar.dma_start(
                    out_d[:, off:off + GROUP], out_sb[:, off:off + GROUP]
                )
            else:
                nc.vector.tensor_copy(out_sb[:, off:off + GROUP], po[:])
                nc.sync.dma_start(
                    out_d[:, off:off + GROUP], out_sb[:, off:off + GROUP]
                )

    nc.finalize()
    return nc


def _get_nc():
    if "nc" not in _NC_CACHE:
        _NC_CACHE["nc"] = _build_bass()
    return _NC_CACHE["nc"]


# ----------------------------------------------------------------------------
# Entry point
# ----------------------------------------------------------------------------

def kernel(input, params):
    global LAST_RESULTS
    from concourse.bass_utils import run_bass_kernel_spmd

    x = np.ascontiguousarray(np.asarray(input, dtype=np.float32))
    p = np.asarray(params, dtype=np.float32)

    w4 = _build_weight_operands(p)
    Z = _round_f32r(_build_Z())

    nc = _get_nc()
    in_maps = []
    for c in range(N_CORES):
        xt = _build_x_operand(x[c * B_CORE:(c + 1) * B_CORE])
        in_maps.append({"xt": xt, "w4": w4, "zt": Z})

    res = run_bass_kernel_spmd(nc, in_maps, list(range(N_CORES)), trace=TRACE)
    LAST_RESULTS = res

    outs = []
    for c in range(N_CORES):
        o = res.results[c]["out"]                 # [11, 2048]
        outs.append((o[:N_OUT] / o[N_OUT:N_OUT + 1]).T)
    return np.ascontiguousarray(np.concatenate(outs, axis=0).astype(np.float32))


# revision 12
# speedup vs baseline: 1.4804x; 1.0464x over previous
"""Trainium2 kernel for nn_Circuit_41936060678727.

The reference is a 10-qubit real-amplitude circuit (CNOT ladders + RY
rotations) applied to an amplitude-embedded batch, measured with PauliZ on
each of the 10 wires.  Every gate is linear in the state, so the whole
8-layer circuit collapses to one fixed 784x1024 matrix W (orthonormal rows)
that depends only on `params`:

    out[b, p] = sum_z (x[b] @ W)[z]^2 * sign_p(z) / sum_z (x[b] @ W)[z]^2

The division makes the pipeline scale-invariant in y = x @ W, which lets the
matmul run in fp8 with generous global scales (SW on W, SX on centered x)
that keep everything out of e4m3's subnormal range.

Device math per core (2048 samples, data-parallel over 8 cores):
    mm1: y^T [1024, 2048] = Waug^T @ xaug         fp8 DoubleRow (0.5 cyc/row)
    sq    = y^2                                    scalar+vector engines, bf16
    mm2: o^T [11, 2048]  = Zsigns^T @ sq           bf16 (1 cyc/row)
Host: out = (o[:10] / o[10])^T, concat cores.

fp8 precision recovery (sim rel err ~1.3e-2 vs 2e-2 gate):
  - x is centered (x - 0.5) so its fp8 error halves; the constant shift is
    restored by 4 "bias rows" (ones on the x side, an fp8 split of
    0.5*colsum(W) on the W side) folded into the contraction for free.
  - 784 main rows pad to 5 DoubleRow chunks of 256 rows; the 496 spare
    slots carry residual-correction rows (W - fp8(W) paired with the same
    x values) that cancel most of the W quantization error.  Chunk 3 is
    the rows-0:256 residual at e5m2 (wide exponent range) and reuses the
    chunk-0 x tile already in SBUF.
"""

import numpy as np
import ml_dtypes

N_QUBITS = 10
DIM = 1 << N_QUBITS          # 1024
N_OUT = 10
D_IN = 784
B_TOTAL = 16384
N_CORES = 8
B_CORE = B_TOTAL // N_CORES  # 2048
GROUP = 512                  # batch columns per matmul (one PSUM bank, fp32)
N_GROUPS = B_CORE // GROUP   # 4
Z_CHUNK = 128
N_ZCH = DIM // Z_CHUNK       # 8
ZCOLS = 16                   # cols 0..9 = PauliZ signs, 10 = ones, 11..15 pad
NCH = 4                      # DoubleRow K-chunks of 256 rows (1024 slots)
SW = 64.0                    # global W scale (pulls W out of e4m3 subnormals)
SX = 4.0                     # global centered-x scale
N_WARM = 28                  # PE warm-up matmuls (clock ramp + DMA prefix)

E4 = ml_dtypes.float8_e4m3
E5 = ml_dtypes.float8_e5m2


# ----------------------------------------------------------------------------
# Host-side precompute: collapse the circuit to W = M[:784, :]
# ----------------------------------------------------------------------------

def _apply_ry(S, theta, q):
    B = S.shape[0]
    left, right = 1 << q, 1 << (N_QUBITS - q - 1)
    s = S.reshape(B, left, 2, right)
    c, sn = np.cos(theta / 2), np.sin(theta / 2)
    s0 = c * s[:, :, 0] - sn * s[:, :, 1]
    s1 = sn * s[:, :, 0] + c * s[:, :, 1]
    return np.stack([s0, s1], axis=2).reshape(B, DIM)


def _apply_cnot(S, q):
    B = S.shape[0]
    left, right = 1 << q, 1 << (N_QUBITS - q - 2)
    s = S.reshape(B, left, 2, 2, right)
    s = np.concatenate([s[:, :, :1], np.flip(s[:, :, 1:], axis=3)], axis=2)
    return s.reshape(B, DIM)


def _build_W(params):
    """Circuit applied to basis rows e_0..e_783 -> W[784, 1024], fp64."""
    w = np.pi * np.tanh(params.astype(np.float64))
    S = np.zeros((D_IN, DIM), dtype=np.float64)
    S[np.arange(D_IN), np.arange(D_IN)] = 1.0
    for l in range(params.shape[0]):
        for start in (0, 1):
            for i in range(start, N_QUBITS - 1, 2):
                S = _apply_cnot(S, i)
        for i in range(N_QUBITS):
            S = _apply_ry(S, w[l, i], i)
    return S


def _build_Z():
    z = np.arange(DIM)
    Z = np.zeros((DIM, ZCOLS), dtype=np.float32)
    for p in range(N_OUT):
        Z[:, p] = 1.0 - 2.0 * ((z >> (N_QUBITS - 1 - p)) & 1)
    Z[:, N_OUT] = 1.0
    # device layout [128, 8*16]: z-chunk c rows c*128..c*128+128 at cols c*16..
    Zd = Z.reshape(N_ZCH, Z_CHUNK, ZCOLS).transpose(1, 0, 2).reshape(Z_CHUNK, -1)
    return np.ascontiguousarray(Zd)


def _q(a, t):
    return np.asarray(a, np.float32).astype(t)


def _chunk_to_tile(A):
    """[256 aug rows, n] -> [128 partitions, 2 halves, n]; slot (p, i) holds
    aug row i*128 + p.  Must match between the W and x sides (it does)."""
    n = A.shape[1]
    return np.ascontiguousarray(A.reshape(2, 128, n).transpose(1, 0, 2))


def _build_weight_operands(params):
    """Returns w4 [8, 128, 4, 256] e4m3."""
    W = _build_W(params)                     # fp64 [784, 1024]
    Ws = W * SW
    Wh = _q(Ws, E4)                          # main fp8 weights
    Wl = Ws - Wh.astype(np.float64)          # residual
    c_s = 0.5 * W.sum(axis=0) * SW * SX      # centering bias, scaled domain
    bias = []
    r = c_s.copy()
    b = _q(r / 2, E4); bias.append(b); r -= b.astype(np.float64)
    for _ in range(3):
        b = _q(r, E4); bias.append(b); r -= b.astype(np.float64)

    # e4m3 chunks in processing order [rows 0:256, 256:512, 512:768, mixed]
    che4 = [
        Wh[0:256], Wh[256:512], Wh[512:768],
        np.concatenate([
            _q(Ws[768:784], E4),             # main tail rows 768..783
            np.stack(bias, axis=0),          # 4 bias rows (x side = ones)
            _q(Wl[0:236], E4),               # residual rows 0..235
        ], axis=0),
    ]

    w4 = np.empty((N_ZCH, 128, 4, 2 * Z_CHUNK), dtype=E4)
    for j, A in enumerate(che4):
        T = _chunk_to_tile(np.ascontiguousarray(A))      # [128, 2, 1024]
        for z in range(N_ZCH):
            blk = T[:, :, z * Z_CHUNK:(z + 1) * Z_CHUNK]  # [128, 2, 128]
            w4[z, :, j, :] = blk.reshape(128, 2 * Z_CHUNK)
    return w4


def _build_x_operand(x_core):
    """x [2048, 784] f32 -> xt [4, 128, 2, 2048] e4m3 (aug chunks 0-2 main,
    chunk 3 = [tail rows 768:784, ones x4, rows 256:492])."""
    xs = (x_core.astype(np.float64) - 0.5) * SX
    xh = _q(xs, E4)                          # [2048, 784]
    xT = np.ascontiguousarray(xh.T)          # [784, 2048]
    ones = np.ones((4, B_CORE), dtype=E4)
    chunks = [
        xT[0:256], xT[256:512], xT[512:768],
        np.concatenate([xT[768:784], ones, xT[0:236]], axis=0),
    ]
    xt = np.empty((4, 128, 2, B_CORE), dtype=E4)
    for c, A in enumerate(chunks):
        xt[c] = _chunk_to_tile(np.ascontiguousarray(A))
    return xt


def _round_f32r(a):
    """fp32 -> float32r encoding (e8m11, RNE): low 12 mantissa bits cleared."""
    u = np.ascontiguousarray(a, dtype=np.float32).view(np.uint32)
    keep = u & np.uint32(0xFFFFF000)
    rem = u & np.uint32(0xFFF)
    inc = (rem > 0x800) | ((rem == 0x800) & (((u >> 12) & 1) == 1))
    out = keep + (inc.astype(np.uint32) << 12)
    return out.view(np.float32)


# ----------------------------------------------------------------------------
# Bass program (identical SPMD program on all 8 cores)
# ----------------------------------------------------------------------------

_NC_CACHE = {}
TRACE = False           # test harness can flip this for profiling
LAST_RESULTS = None


def _build_bass():
    from contextlib import ExitStack

    import concourse.tile as tile
    from concourse import bacc, mybir

    f32 = mybir.dt.float32
    f32r = mybir.dt.float32r
    f8e4 = mybir.dt.float8e4
    f8e5 = mybir.dt.float8e5
    bf16 = mybir.dt.bfloat16
    DR = mybir.MatmulPerfMode.DoubleRow
    MULT = mybir.AluOpType.mult

    nc = bacc.Bacc(
        "TRN2", target_bir_lowering=False, debug=False, num_devices=N_CORES
    )
    xt_d = nc.declare_dram_parameter("xt", [4, 128, 2, B_CORE], f8e4, isOutput=False)
    w4_d = nc.declare_dram_parameter("w4", [N_ZCH, 128, 4, 256], f8e4, isOutput=False)
    zt_d = nc.declare_dram_parameter("zt", [Z_CHUNK, N_ZCH * ZCOLS], f32r, isOutput=False)
    out_d = nc.declare_dram_parameter("out", [N_OUT + 1, B_CORE], f32, isOutput=True)

    N_PH = 4                      # column phases of one 512-col group each

    with ExitStack() as ctx:
        tc = ctx.enter_context(tile.TileContext(nc))
        wpool = ctx.enter_context(tc.tile_pool(name="w", bufs=1))
        xpool = ctx.enter_context(tc.tile_pool(name="x", bufs=1))
        zpool = ctx.enter_context(tc.tile_pool(name="z", bufs=1))
        sqpool = ctx.enter_context(tc.tile_pool(name="sq", bufs=3))
        opool = ctx.enter_context(tc.tile_pool(name="osb", bufs=1))
        pypool = ctx.enter_context(tc.tile_pool(name="py", bufs=2, space="PSUM"))
        popool = ctx.enter_context(tc.tile_pool(name="po", bufs=1, space="PSUM"))

        # PE pre-warm: K=128 bf16 matmuls with the array fully lit -- the
        # HAM/DVFS clock ramps on real array activity (K=1 warms leave it at
        # 1.2 GHz), and the warm window also covers the first x slab's DMA.
        warm_in = opool.tile([128, 384], bf16, name="warm_in")
        nc.any.memset(warm_in[:], 1.0)
        warm_ps = popool.tile([128, 256], f32, name="warm_ps", tag="warm")
        for _ in range(N_WARM):
            nc.tensor.matmul(
                warm_ps[:],
                lhsT=warm_in[:, 0:128],
                rhs=warm_in[:, 128:384],
                start=True,
                stop=True,
                skip_group_check=True,
            )

        # Input DMAs, split over two rings (sync + gpsimd) so the phase-0
        # critical set (signs, z0 weights, first x quarter) streams first.
        z_sb = zpool.tile([Z_CHUNK, N_ZCH * ZCOLS], f32r)
        nc.sync.dma_start(z_sb[:], zt_d[:, :])
        w4_sb = [None] * N_ZCH
        x_sb = [[None] * N_PH for _ in range(4)]

        def load_w(z):
            t = wpool.tile([128, 4, 2, Z_CHUNK], f8e4, tag=f"w4_{z}", name=f"w4_{z}")
            nc.sync.dma_start(t[:], w4_d[z, :, :, :])
            w4_sb[z] = t

        def load_x(c, q, eng):
            t = xpool.tile([128, 2, GROUP], f8e4, tag=f"x{c}q{q}", name=f"x{c}q{q}")
            eng.dma_start(t[:], xt_d[c, :, :, q * GROUP:(q + 1) * GROUP])
            x_sb[c][q] = t

        load_w(0)
        load_x(0, 0, nc.sync)
        load_x(1, 0, nc.sync)
        load_w(1)
        load_w(2)
        load_x(0, 1, nc.sync)
        load_x(1, 1, nc.sync)
        load_w(3)
        load_w(4)
        load_x(0, 2, nc.sync)
        load_x(1, 2, nc.sync)
        load_w(5)
        load_w(6)
        load_w(7)
        load_x(0, 3, nc.sync)
        load_x(1, 3, nc.sync)
        for q in range(N_PH):
            for c in (2, 3):
                load_x(c, q, nc.gpsimd)

        out_sb = opool.tile([N_OUT + 1, B_CORE], f32)

        for h in range(N_PH):
            po = popool.tile(
                [N_OUT + 1, GROUP], f32, tag=f"po{h % 2}", name=f"po_{h}"
            )

            def emit_mm2(sq, z):
                nc.tensor.matmul(
                    po[:],
                    lhsT=z_sb[:, z * ZCOLS:z * ZCOLS + N_OUT + 1],
                    rhs=sq[:],
                    start=(z == 0),
                    stop=(z == N_ZCH - 1),
                    skip_group_check=True,
                )

            sqs = []
            for z in range(N_ZCH):
                py = pypool.tile(
                    [Z_CHUNK, GROUP], f32, tag="py", name=f"py_{h}_{z}"
                )
                for ci in range(4):
                    nc.tensor.matmul(
                        py[:],
                        lhsT=w4_sb[z][:, ci],
                        rhs=x_sb[ci][h][:],
                        start=(ci == 0),
                        stop=(ci == NCH - 1),
                        perf_mode=DR,
                        skip_group_check=True,
                    )
                # sign-contraction runs two z behind mm1, so it never waits
                # on the square's engine latency
                if z >= 2:
                    emit_mm2(sqs[z - 2], z - 2)
                sq = sqpool.tile(
                    [Z_CHUNK, GROUP], f32r, tag="sq", name=f"sq_{h}_{z}"
                )
                nc.scalar.square(sq[:], py[:])
                sqs.append(sq)
            emit_mm2(sqs[N_ZCH - 2], N_ZCH - 2)
            emit_mm2(sqs[N_ZCH - 1], N_ZCH - 1)
            off = h * GROUP
            if h % 2 == 0:
                nc.scalar.copy(out_sb[:, off:off + GROUP], po[:])
                nc.scalar.dma_start(
                    out_d[:, off:off + GROUP], out_sb[:, off:off + GROUP]
                )
            else:
                nc.vector.tensor_copy(out_sb[:, off:off + GROUP], po[:])
                nc.sync.dma_start(
                    out_d[:, off:off + GROUP], out_sb[:, off:off + GROUP]
                )

    nc.finalize()
    return nc


def _get_nc():
    if "nc" not in _NC_CACHE:
        _NC_CACHE["nc"] = _build_bass()
    return _NC_CACHE["nc"]


# ----------------------------------------------------------------------------
# Entry point
# ----------------------------------------------------------------------------

def kernel(input, params):
    global LAST_RESULTS
    from concourse.bass_utils import run_bass_kernel_spmd

    x = np.ascontiguousarray(np.asarray(input, dtype=np.float32))
    p = np.asarray(params, dtype=np.float32)

    w4 = _build_weight_operands(p)
    Z = _round_f32r(_build_Z())

    nc = _get_nc()
    in_maps = []
    for c in range(N_CORES):
        xt = _build_x_operand(x[c * B_CORE:(c + 1) * B_CORE])
        in_maps.append({"xt": xt, "w4": w4, "zt": Z})

    res = run_bass_kernel_spmd(nc, in_maps, list(range(N_CORES)), trace=TRACE)
    LAST_RESULTS = res

    outs = []
    for c in range(N_CORES):
        o = res.results[c]["out"]                 # [11, 2048]
        outs.append((o[:N_OUT] / o[N_OUT:N_OUT + 1]).T)
    return np.ascontiguousarray(np.concatenate(outs, axis=0).astype(np.float32))


# revision 13
# speedup vs baseline: 1.5050x; 1.0166x over previous
"""Trainium2 kernel for nn_Circuit_41936060678727.

The reference is a 10-qubit real-amplitude circuit (CNOT ladders + RY
rotations) applied to an amplitude-embedded batch, measured with PauliZ on
each of the 10 wires.  Every gate is linear in the state, so the whole
8-layer circuit collapses to one fixed 784x1024 matrix W (orthonormal rows)
that depends only on `params`:

    out[b, p] = sum_z (x[b] @ W)[z]^2 * sign_p(z) / sum_z (x[b] @ W)[z]^2

The division makes the pipeline scale-invariant in y = x @ W, which lets the
matmul run in fp8 with generous global scales (SW on W, SX on centered x)
that keep everything out of e4m3's subnormal range.

Device math per core (2048 samples, data-parallel over 8 cores):
    mm1: y^T [1024, 2048] = Waug^T @ xaug         fp8 DoubleRow (0.5 cyc/row)
    sq    = y^2                                    scalar+vector engines, bf16
    mm2: o^T [11, 2048]  = Zsigns^T @ sq           bf16 (1 cyc/row)
Host: out = (o[:10] / o[10])^T, concat cores.

fp8 precision recovery (sim rel err ~1.3e-2 vs 2e-2 gate):
  - x is centered (x - 0.5) so its fp8 error halves; the constant shift is
    restored by 4 "bias rows" (ones on the x side, an fp8 split of
    0.5*colsum(W) on the W side) folded into the contraction for free.
  - 784 main rows pad to 5 DoubleRow chunks of 256 rows; the 496 spare
    slots carry residual-correction rows (W - fp8(W) paired with the same
    x values) that cancel most of the W quantization error.  Chunk 3 is
    the rows-0:256 residual at e5m2 (wide exponent range) and reuses the
    chunk-0 x tile already in SBUF.
"""

import numpy as np
import ml_dtypes

N_QUBITS = 10
DIM = 1 << N_QUBITS          # 1024
N_OUT = 10
D_IN = 784
B_TOTAL = 16384
N_CORES = 8
B_CORE = B_TOTAL // N_CORES  # 2048
GROUP = 512                  # batch columns per matmul (one PSUM bank, fp32)
N_GROUPS = B_CORE // GROUP   # 4
Z_CHUNK = 128
N_ZCH = DIM // Z_CHUNK       # 8
ZCOLS = 16                   # cols 0..9 = PauliZ signs, 10 = ones, 11..15 pad
NCH = 4                      # DoubleRow K-chunks of 256 rows (1024 slots)
SW = 64.0                    # global W scale (pulls W out of e4m3 subnormals)
SX = 4.0                     # global centered-x scale
N_WARM = 22                  # PE warm-up matmuls (clock ramp + DMA prefix)

E4 = ml_dtypes.float8_e4m3
E5 = ml_dtypes.float8_e5m2


# ----------------------------------------------------------------------------
# Host-side precompute: collapse the circuit to W = M[:784, :]
# ----------------------------------------------------------------------------

def _apply_ry(S, theta, q):
    B = S.shape[0]
    left, right = 1 << q, 1 << (N_QUBITS - q - 1)
    s = S.reshape(B, left, 2, right)
    c, sn = np.cos(theta / 2), np.sin(theta / 2)
    s0 = c * s[:, :, 0] - sn * s[:, :, 1]
    s1 = sn * s[:, :, 0] + c * s[:, :, 1]
    return np.stack([s0, s1], axis=2).reshape(B, DIM)


def _apply_cnot(S, q):
    B = S.shape[0]
    left, right = 1 << q, 1 << (N_QUBITS - q - 2)
    s = S.reshape(B, left, 2, 2, right)
    s = np.concatenate([s[:, :, :1], np.flip(s[:, :, 1:], axis=3)], axis=2)
    return s.reshape(B, DIM)


def _build_W(params):
    """Circuit applied to basis rows e_0..e_783 -> W[784, 1024], fp64."""
    w = np.pi * np.tanh(params.astype(np.float64))
    S = np.zeros((D_IN, DIM), dtype=np.float64)
    S[np.arange(D_IN), np.arange(D_IN)] = 1.0
    for l in range(params.shape[0]):
        for start in (0, 1):
            for i in range(start, N_QUBITS - 1, 2):
                S = _apply_cnot(S, i)
        for i in range(N_QUBITS):
            S = _apply_ry(S, w[l, i], i)
    return S


def _build_Z():
    z = np.arange(DIM)
    Z = np.zeros((DIM, ZCOLS), dtype=np.float32)
    for p in range(N_OUT):
        Z[:, p] = 1.0 - 2.0 * ((z >> (N_QUBITS - 1 - p)) & 1)
    Z[:, N_OUT] = 1.0
    # device layout [128, 8*16]: z-chunk c rows c*128..c*128+128 at cols c*16..
    Zd = Z.reshape(N_ZCH, Z_CHUNK, ZCOLS).transpose(1, 0, 2).reshape(Z_CHUNK, -1)
    return np.ascontiguousarray(Zd)


def _q(a, t):
    return np.asarray(a, np.float32).astype(t)


def _chunk_to_tile(A):
    """[256 aug rows, n] -> [128 partitions, 2 halves, n]; slot (p, i) holds
    aug row i*128 + p.  Must match between the W and x sides (it does)."""
    n = A.shape[1]
    return np.ascontiguousarray(A.reshape(2, 128, n).transpose(1, 0, 2))


def _build_weight_operands(params):
    """Returns w4 [8, 128, 4, 256] e4m3."""
    W = _build_W(params)                     # fp64 [784, 1024]
    Ws = W * SW
    Wh = _q(Ws, E4)                          # main fp8 weights
    Wl = Ws - Wh.astype(np.float64)          # residual
    c_s = 0.5 * W.sum(axis=0) * SW * SX      # centering bias, scaled domain
    bias = []
    r = c_s.copy()
    b = _q(r / 2, E4); bias.append(b); r -= b.astype(np.float64)
    for _ in range(3):
        b = _q(r, E4); bias.append(b); r -= b.astype(np.float64)

    # e4m3 chunks in processing order [rows 0:256, 256:512, 512:768, mixed]
    che4 = [
        Wh[0:256], Wh[256:512], Wh[512:768],
        np.concatenate([
            _q(Ws[768:784], E4),             # main tail rows 768..783
            np.stack(bias, axis=0),          # 4 bias rows (x side = ones)
            _q(Wl[0:236], E4),               # residual rows 0..235
        ], axis=0),
    ]

    w4 = np.empty((N_ZCH, 128, 4, 2 * Z_CHUNK), dtype=E4)
    for j, A in enumerate(che4):
        T = _chunk_to_tile(np.ascontiguousarray(A))      # [128, 2, 1024]
        for z in range(N_ZCH):
            blk = T[:, :, z * Z_CHUNK:(z + 1) * Z_CHUNK]  # [128, 2, 128]
            w4[z, :, j, :] = blk.reshape(128, 2 * Z_CHUNK)
    return w4


def _build_x_operand(x_core):
    """x [2048, 784] f32 -> xt [4, 128, 2, 2048] e4m3 (aug chunks 0-2 main,
    chunk 3 = [tail rows 768:784, ones x4, rows 256:492])."""
    xs = (x_core.astype(np.float64) - 0.5) * SX
    xh = _q(xs, E4)                          # [2048, 784]
    xT = np.ascontiguousarray(xh.T)          # [784, 2048]
    ones = np.ones((4, B_CORE), dtype=E4)
    chunks = [
        xT[0:256], xT[256:512], xT[512:768],
        np.concatenate([xT[768:784], ones, xT[0:236]], axis=0),
    ]
    xt = np.empty((4, 128, 2, B_CORE), dtype=E4)
    for c, A in enumerate(chunks):
        xt[c] = _chunk_to_tile(np.ascontiguousarray(A))
    return xt


def _round_f32r(a):
    """fp32 -> float32r encoding (e8m11, RNE): low 12 mantissa bits cleared."""
    u = np.ascontiguousarray(a, dtype=np.float32).view(np.uint32)
    keep = u & np.uint32(0xFFFFF000)
    rem = u & np.uint32(0xFFF)
    inc = (rem > 0x800) | ((rem == 0x800) & (((u >> 12) & 1) == 1))
    out = keep + (inc.astype(np.uint32) << 12)
    return out.view(np.float32)


# ----------------------------------------------------------------------------
# Bass program (identical SPMD program on all 8 cores)
# ----------------------------------------------------------------------------

_NC_CACHE = {}
TRACE = False           # test harness can flip this for profiling
LAST_RESULTS = None


def _build_bass():
    from contextlib import ExitStack

    import concourse.tile as tile
    from concourse import bacc, mybir

    f32 = mybir.dt.float32
    f32r = mybir.dt.float32r
    f8e4 = mybir.dt.float8e4
    f8e5 = mybir.dt.float8e5
    bf16 = mybir.dt.bfloat16
    DR = mybir.MatmulPerfMode.DoubleRow
    MULT = mybir.AluOpType.mult

    nc = bacc.Bacc(
        "TRN2", target_bir_lowering=False, debug=False, num_devices=N_CORES
    )
    xt_d = nc.declare_dram_parameter("xt", [4, 128, 2, B_CORE], f8e4, isOutput=False)
    w4_d = nc.declare_dram_parameter("w4", [N_ZCH, 128, 4, 256], f8e4, isOutput=False)
    zt_d = nc.declare_dram_parameter("zt", [Z_CHUNK, N_ZCH * ZCOLS], f32r, isOutput=False)
    out_d = nc.declare_dram_parameter("out", [N_OUT + 1, B_CORE], f32, isOutput=True)

    N_PH = 4                      # column phases of one 512-col group each

    with ExitStack() as ctx:
        tc = ctx.enter_context(tile.TileContext(nc))
        wpool = ctx.enter_context(tc.tile_pool(name="w", bufs=1))
        xpool = ctx.enter_context(tc.tile_pool(name="x", bufs=1))
        zpool = ctx.enter_context(tc.tile_pool(name="z", bufs=1))
        sqpool = ctx.enter_context(tc.tile_pool(name="sq", bufs=3))
        opool = ctx.enter_context(tc.tile_pool(name="osb", bufs=1))
        pypool = ctx.enter_context(tc.tile_pool(name="py", bufs=2, space="PSUM"))
        popool = ctx.enter_context(tc.tile_pool(name="po", bufs=1, space="PSUM"))

        # PE pre-warm: K=128 bf16 matmuls with the array fully lit -- the
        # HAM/DVFS clock ramps on real array activity (K=1 warms leave it at
        # 1.2 GHz), and the warm window also covers the first x slab's DMA.
        warm_in = opool.tile([128, 384], bf16, name="warm_in")
        nc.any.memset(warm_in[:], 1.0)
        warm_ps = popool.tile([128, 256], f32, name="warm_ps", tag="warm")
        for _ in range(N_WARM):
            nc.tensor.matmul(
                warm_ps[:],
                lhsT=warm_in[:, 0:128],
                rhs=warm_in[:, 128:384],
                start=True,
                stop=True,
                skip_group_check=True,
            )

        # Input DMAs, split over two rings (sync + gpsimd) so the phase-0
        # critical set (signs, z0 weights, first x quarter) streams first.
        z_sb = zpool.tile([Z_CHUNK, N_ZCH * ZCOLS], f32r)
        nc.sync.dma_start(z_sb[:], zt_d[:, :])
        w4_sb = [None] * N_ZCH
        x_sb = [[None] * N_PH for _ in range(4)]

        def load_w(z):
            t = wpool.tile([128, 4, 2, Z_CHUNK], f8e4, tag=f"w4_{z}", name=f"w4_{z}")
            nc.sync.dma_start(t[:], w4_d[z, :, :, :])
            w4_sb[z] = t

        def load_x(c, q, eng):
            t = xpool.tile([128, 2, GROUP], f8e4, tag=f"x{c}q{q}", name=f"x{c}q{q}")
            eng.dma_start(t[:], xt_d[c, :, :, q * GROUP:(q + 1) * GROUP])
            x_sb[c][q] = t

        load_w(0)
        load_x(0, 0, nc.sync)
        load_w(1)
        load_x(0, 1, nc.sync)
        load_w(2)
        load_w(3)
        load_x(0, 2, nc.sync)
        load_w(4)
        load_w(5)
        load_x(0, 3, nc.sync)
        load_w(6)
        load_w(7)
        for q in range(N_PH):
            for c in (1, 2, 3):
                load_x(c, q, nc.gpsimd)

        out_sb = opool.tile([N_OUT + 1, B_CORE], f32)

        for h in range(N_PH):
            po = popool.tile(
                [N_OUT + 1, GROUP], f32, tag=f"po{h % 2}", name=f"po_{h}"
            )

            def emit_mm2(sq, z):
                nc.tensor.matmul(
                    po[:],
                    lhsT=z_sb[:, z * ZCOLS:z * ZCOLS + N_OUT + 1],
                    rhs=sq[:],
                    start=(z == 0),
                    stop=(z == N_ZCH - 1),
                    skip_group_check=True,
                )

            sqs = []
            for z in range(N_ZCH):
                py = pypool.tile(
                    [Z_CHUNK, GROUP], f32, tag="py", name=f"py_{h}_{z}"
                )
                for ci in range(4):
                    nc.tensor.matmul(
                        py[:],
                        lhsT=w4_sb[z][:, ci],
                        rhs=x_sb[ci][h][:],
                        start=(ci == 0),
                        stop=(ci == NCH - 1),
                        perf_mode=DR,
                        skip_group_check=True,
                    )
                # sign-contraction runs two z behind mm1, so it never waits
                # on the square's engine latency
                if z >= 2:
                    emit_mm2(sqs[z - 2], z - 2)
                sq = sqpool.tile(
                    [Z_CHUNK, GROUP], f32r, tag="sq", name=f"sq_{h}_{z}"
                )
                nc.scalar.square(sq[:], py[:])
                sqs.append(sq)
            emit_mm2(sqs[N_ZCH - 2], N_ZCH - 2)
            emit_mm2(sqs[N_ZCH - 1], N_ZCH - 1)
            off = h * GROUP
            nc.vector.tensor_copy(out_sb[:, off:off + GROUP], po[:])
            nc.sync.dma_start(
                out_d[:, off:off + GROUP], out_sb[:, off:off + GROUP]
            )

    nc.finalize()
    return nc


def _get_nc():
    if "nc" not in _NC_CACHE:
        _NC_CACHE["nc"] = _build_bass()
    return _NC_CACHE["nc"]


# ----------------------------------------------------------------------------
# Entry point
# ----------------------------------------------------------------------------

def kernel(input, params):
    global LAST_RESULTS
    from concourse.bass_utils import run_bass_kernel_spmd

    x = np.ascontiguousarray(np.asarray(input, dtype=np.float32))
    p = np.asarray(params, dtype=np.float32)

    w4 = _build_weight_operands(p)
    Z = _round_f32r(_build_Z())

    nc = _get_nc()
    in_maps = []
    for c in range(N_CORES):
        xt = _build_x_operand(x[c * B_CORE:(c + 1) * B_CORE])
        in_maps.append({"xt": xt, "w4": w4, "zt": Z})

    res = run_bass_kernel_spmd(nc, in_maps, list(range(N_CORES)), trace=TRACE)
    LAST_RESULTS = res

    outs = []
    for c in range(N_CORES):
        o = res.results[c]["out"]                 # [11, 2048]
        outs.append((o[:N_OUT] / o[N_OUT:N_OUT + 1]).T)
    return np.ascontiguousarray(np.concatenate(outs, axis=0).astype(np.float32))
